# revision 1
# baseline (speedup 1.0000x reference)
"""MemoryRetriever kernel for 8x Trainium2 NeuronCores.

Data-parallel over the B*S=8192 query rows (1024 rows/core); the selected
memory bank and all weights are replicated. All heavy matmuls run in fp32r
(fp32 rounded to 11-bit mantissa, 1 PE cycle/row at free-dim 512).

Device activations live feature-major: [feature partition chunks of 128, rows].

Host-side linear-algebra fusions (exact up to fp32 rounding):
  Q = x @ (wq_in @ Wq).T + (wq_in @ bq + bqi)
  mem-layernorm gamma/beta are folded into wk/wv/bk/bv
  attn_out = ctx @ out_w.T + out_b is folded into the gate/integration
  weights:  cat @ W.T = x @ Wx.T + ctx @ (Wa @ out_w).T + (Wa @ out_b + b)
so the device never materializes attn_out; cat == [x; ctx].

Weights are passed in chunk-contiguous layout [OUTC, 128, INC, ow] so every
weight DMA reads 4-8KB contiguous per partition.
"""

import sys
from contextlib import ExitStack

if "/opt/trn_rl_repo" not in sys.path:
    sys.path.insert(0, "/opt/trn_rl_repo")

import numpy as np

import concourse.bass as bass
import concourse.mybir as mybir
import concourse.tile as tile
from concourse import bacc
from concourse.bass_utils import run_bass_kernel_spmd
from concourse.masks import make_identity

F32 = mybir.dt.float32
F32R = mybir.dt.float32r
AF = mybir.ActivationFunctionType
OP = mybir.AluOpType

H = 1024
NH = 4
HD = H // NH          # 256
K = 2048              # top_k
B, S = 4, 2048
N_CORES = 8
R = (B * S) // N_CORES  # 1024 rows per core
EPS = 1e-5
H2 = 2 * H            # 2048

HC = H // 128         # 8 feature chunks
H2C = H2 // 128       # 16
KC = K // 128         # 16 key chunks
RT = R // 512         # 2 row tiles of 512
KT4 = K // 512        # 4 key tiles of 512


def build_program():
    nc = bacc.Bacc("TRN2", target_bir_lowering=False)

    d_xt = nc.declare_dram_parameter("x_t", [H, R], F32R, isOutput=False)
    d_mem = nc.declare_dram_parameter("mem_t", [H, K], F32R, isOutput=False)
    d_wc = nc.declare_dram_parameter("wc_t", [HC, 128, HC, 128], F32R,
                                     isOutput=False)
    d_wk = nc.declare_dram_parameter("wk_t", [HC, 128, HC, 128], F32R,
                                     isOutput=False)
    d_wv = nc.declare_dram_parameter("wv_t", [2, 128, HC, 512], F32R,
                                     isOutput=False)
    d_gw = nc.declare_dram_parameter("gw_t", [HC, 128, H2C, 128], F32R,
                                     isOutput=False)
    d_w1 = nc.declare_dram_parameter("w1_t", [H2C, 128, H2C, 128], F32R,
                                     isOutput=False)
    d_w2 = nc.declare_dram_parameter("w2_t", [HC, 128, H2C, 128], F32R,
                                     isOutput=False)
    d_bc = nc.declare_dram_parameter("bc", [H], F32, isOutput=False)
    d_bk = nc.declare_dram_parameter("bk", [H], F32, isOutput=False)
    d_bv = nc.declare_dram_parameter("bv", [H], F32, isOutput=False)
    d_gb = nc.declare_dram_parameter("gate_b", [H], F32, isOutput=False)
    d_b1 = nc.declare_dram_parameter("int_b1", [H2], F32, isOutput=False)
    d_b2 = nc.declare_dram_parameter("int_b2", [H], F32, isOutput=False)
    d_ilg = nc.declare_dram_parameter("iln_g", [H2], F32, isOutput=False)
    d_ilb = nc.declare_dram_parameter("iln_b", [H2], F32, isOutput=False)
    d_l2g = nc.declare_dram_parameter("ln2_g", [H], F32, isOutput=False)
    d_l2b = nc.declare_dram_parameter("ln2_b", [H], F32, isOutput=False)
    d_out = nc.declare_dram_parameter("out", [R, H], F32, isOutput=True)

    # DRAM roundtrip for K/V (SBUF can't hold them alongside everything else)
    d_ktd = nc.dram_tensor("ktD", [H, K], F32R)
    d_vd = nc.dram_tensor("vD", [K, H], F32R)

    with tile.TileContext(nc) as tc, ExitStack() as top:
        singles = top.enter_context(tc.tile_pool(name="singles", bufs=1))

        ident = singles.tile([128, 128], F32)
        make_identity(nc, ident)
        scratch1 = singles.tile([128, 128], F32)
        nc.vector.memset(scratch1, 1.0)
        ones_sm = singles.tile([128, 128], F32R)
        nc.scalar.activation(out=ones_sm, in_=scratch1, func=AF.Copy)
        ones_1k = singles.tile([128, 128], F32R)
        nc.scalar.activation(out=ones_1k, in_=scratch1, func=AF.Copy,
                             scale=1.0 / 1024.0)
        ones_2k = singles.tile([128, 128], F32R)
        nc.scalar.activation(out=ones_2k, in_=scratch1, func=AF.Copy,
                             scale=1.0 / 2048.0)
        eps_t = singles.tile([128, 1], F32)
        nc.vector.memset(eps_t, EPS)

        def load_pp(vec, n, nm):  # [n*128] dram vector -> [128, n] per-partition
            t = singles.tile([128, n], F32, tag=f"pp_{nm}", name=f"pp_{nm}")
            nc.sync.dma_start(out=t, in_=vec[:].rearrange("(c p) -> p c", p=128))
            return t

        # =========== Phase A: mem layernorm + K/V projections ===========
        with ExitStack() as sa:
            pa = sa.enter_context(tc.tile_pool(name="pa", bufs=1))
            pa_sq = sa.enter_context(tc.tile_pool(name="pa_sq", bufs=2))
            mem_sb = pa.tile([128, HC, K], F32R)  # 8 MB
            for hc in range(HC):
                nc.sync.dma_start(out=mem_sb[:, hc, :],
                                  in_=d_mem[hc * 128:(hc + 1) * 128, :])
            mu_sb = pa.tile([128, K], F32)
            rstd_sb = pa.tile([128, K], F32)
            bc_sb = load_pp(d_bc, HC, "bc")
            bk_sb = load_pp(d_bk, HC, "bk")
            bv_sb = load_pp(d_bv, HC, "bv")
            gb_sb = load_pp(d_gb, HC, "gb")
            b1_sb = load_pp(d_b1, H2C, "b1")
            b2_sb = load_pp(d_b2, HC, "b2")
            ilg_sb = load_pp(d_ilg, H2C, "ilg")
            ilb_sb = load_pp(d_ilb, H2C, "ilb")
            # stats: mean / mean-square over the 1024 features (partition dim)
            with tc.tile_pool(name="pa_st", bufs=1, space="PSUM") as pa_st:
                mu_ps = [pa_st.tile([128, 512], F32, tag=f"mu{i}", name=f"mu{i}")
                         for i in range(KT4)]
                ms_ps = [pa_st.tile([128, 512], F32, tag=f"ms{i}", name=f"ms{i}")
                         for i in range(KT4)]
                for hc in range(HC):
                    sq = pa_sq.tile([128, K], F32R, tag="sqt1", name="sq")
                    nc.vector.tensor_mul(sq, mem_sb[:, hc, :].bitcast(F32),
                                         mem_sb[:, hc, :].bitcast(F32))
                    for i in range(KT4):
                        sl = bass.ts(i, 512)
                        nc.tensor.matmul(mu_ps[i], ones_1k, mem_sb[:, hc, sl],
                                         start=(hc == 0), stop=(hc == HC - 1))
                        nc.tensor.matmul(ms_ps[i], ones_1k, sq[:, sl],
                                         start=(hc == 0), stop=(hc == HC - 1))
                for i in range(KT4):
                    sl = bass.ts(i, 512)
                    nc.scalar.activation(out=mu_sb[:, sl], in_=mu_ps[i],
                                         func=AF.Copy)
                    var = pa_sq.tile([128, 512], F32, tag="var", name="var")
                    nc.vector.tensor_mul(var, mu_sb[:, sl], mu_sb[:, sl])
                    nc.vector.tensor_sub(var, ms_ps[i], var)
                    # rstd = exp(-0.5*ln(var+eps)); Ln/Exp share one table set
                    nc.scalar.activation(out=var, in_=var, func=AF.Ln,
                                         bias=eps_t, scale=1.0)
                    nc.scalar.activation(out=rstd_sb[:, sl], in_=var,
                                         func=AF.Exp, scale=-0.5)
            # apply LN in place (f32r); ln1 gamma/beta folded into wk/wv on host
            # per 512-wide tile so the K projection can start on tile 0 early
            for i in range(KT4):
                sl = bass.ts(i, 512)
                for hc in range(HC):
                    t1 = pa_sq.tile([128, 512], F32, tag="sqt1", name="t1")
                    nc.vector.tensor_sub(t1, mem_sb[:, hc, sl].bitcast(F32),
                                         mu_sb[:, sl])
                    nc.vector.tensor_mul(mem_sb[:, hc, sl], t1, rstd_sb[:, sl])
            # K_t = wk.T-matmul(mem_n) + bk  -> dram ktD [H, K]
            with ExitStack() as skv:
                pa_w = skv.enter_context(tc.tile_pool(name="pa_w", bufs=2))
                pa_o = skv.enter_context(tc.tile_pool(name="pa_o", bufs=4))
                pa_ps = skv.enter_context(
                    tc.tile_pool(name="pa_ps", bufs=2, space="PSUM"))
                for oc in range(HC):
                    wks = pa_w.tile([128, HC, 128], F32R, tag="wk", name="wks", bufs=3)
                    nc.sync.dma_start(out=wks, in_=d_wk[oc])
                    for i in range(KT4):
                        sl = bass.ts(i, 512)
                        ps = pa_ps.tile([128, 512], F32, tag="kps", name="kps")
                        for hc in range(HC):
                            nc.tensor.matmul(ps, wks[:, hc, :], mem_sb[:, hc, sl],
                                             start=(hc == 0), stop=(hc == HC - 1))
                        ko = pa_o.tile([128, 512], F32R, tag="ko", name="ko")
                        nc.scalar.activation(out=ko, in_=ps, func=AF.Identity,
                                             bias=bk_sb[:, oc:oc + 1])
                        nc.sync.dma_start(out=d_ktd[oc * 128:(oc + 1) * 128, sl],
                                          in_=ko)
                # V = mem_n @ wv.T (bias bv folded after softmax) -> dram vD [K,H]
                for ot in range(2):
                    osl = bass.ts(ot, 512)
                    wvs = pa_w.tile([128, HC, 512], F32R, tag="wv", name="wvs")
                    nc.sync.dma_start(out=wvs, in_=d_wv[ot])
                    for kc in range(KC):
                        ps = pa_ps.tile([128, 512], F32, tag="vps", name="vps")
                        for hc in range(HC):
                            nc.tensor.matmul(
                                ps, mem_sb[:, hc, kc * 128:(kc + 1) * 128],
                                wvs[:, hc, :],
                                start=(hc == 0), stop=(hc == HC - 1))
                        vo = pa_o.tile([128, 512], F32R, tag="vo", name="vo")
                        nc.scalar.activation(out=vo, in_=ps, func=AF.Copy)
                        nc.sync.dma_start(
                            out=d_vd[kc * 128:(kc + 1) * 128, osl], in_=vo)

        xt_sb = singles.tile([128, HC, R], F32R)   # resident until the end
        for hc in range(HC):
            nc.sync.dma_start(out=xt_sb[:, hc, :],
                              in_=d_xt[hc * 128:(hc + 1) * 128, :])

        # =========== Phases B+C: query projection + attention ===========
        with ExitStack() as sbc:
            pct = sbc.enter_context(tc.tile_pool(name="pct", bufs=1))
            ctxt_sb = pct.tile([128, HC, R], F32R)
            with ExitStack() as spq:
                pq = spq.enter_context(tc.tile_pool(name="pq", bufs=1))
                qt_sb = pq.tile([128, HC, R], F32R)
                with ExitStack() as sb_:
                    pb_w = sb_.enter_context(tc.tile_pool(name="pb_w", bufs=3))
                    pb_ps = sb_.enter_context(
                        tc.tile_pool(name="pb_ps", bufs=4, space="PSUM"))
                    for oc in range(HC):
                        wcs = pb_w.tile([128, HC, 128], F32R, tag="wc",
                                        name="wcs")
                        nc.sync.dma_start(out=wcs, in_=d_wc[oc])
                        for rt in range(RT):
                            sl = bass.ts(rt, 512)
                            ps = pb_ps.tile([128, 512], F32, tag="qps",
                                            name="qps")
                            for hc in range(HC):
                                nc.tensor.matmul(ps, wcs[:, hc, :],
                                                 xt_sb[:, hc, sl],
                                                 start=(hc == 0),
                                                 stop=(hc == HC - 1))
                            nc.scalar.activation(out=qt_sb[:, oc, sl], in_=ps,
                                                 func=AF.Identity,
                                                 bias=bc_sb[:, oc:oc + 1])

                with ExitStack() as sc_:
                    pc_kv = sc_.enter_context(tc.tile_pool(name="pc_kv", bufs=2))
                    pc_e = sc_.enter_context(tc.tile_pool(name="pc_e", bufs=6))
                    pc_o = sc_.enter_context(tc.tile_pool(name="pc_o", bufs=4))
                    pc_sc = sc_.enter_context(
                        tc.tile_pool(name="pc_sc", bufs=2, space="PSUM"))
                    pc_acc = sc_.enter_context(
                        tc.tile_pool(name="pc_acc", bufs=2, space="PSUM"))
                    for h in range(NH):
                        kh = pc_kv.tile([128, 2, K], F32R, tag="kh", name="kh")
                        for j in range(2):
                            row0 = h * HD + j * 128
                            nc.sync.dma_start(out=kh[:, j, :],
                                              in_=d_ktd[row0:row0 + 128, :])
                        vh = pc_kv.tile([128, KC, HD], F32R, tag="vh", name="vh")
                        for kc in range(KC):
                            nc.sync.dma_start(
                                out=vh[:, kc, :],
                                in_=d_vd[kc * 128:(kc + 1) * 128,
                                         h * HD:(h + 1) * HD])
                        for qt in range(RT):
                            qsl = bass.ts(qt, 512)
                            sums = pc_acc.tile([128, 512], F32, tag="sums",
                                               name="sums")
                            ctx0 = pc_acc.tile([128, 512], F32, tag="ctx0",
                                               name="ctx0")
                            ctx1 = pc_acc.tile([128, 512], F32, tag="ctx1",
                                               name="ctx1")
                            for kt in range(KC):
                                sc = pc_sc.tile([128, 512], F32, tag="sc",
                                                name="sc")
                                for j in range(2):
                                    nc.tensor.matmul(
                                        sc, kh[:, j, kt * 128:(kt + 1) * 128],
                                        qt_sb[:, h * 2 + j, qsl],
                                        start=(j == 0), stop=(j == 1))
                                e = pc_e.tile([128, 512], F32R, tag="e",
                                              name="e")
                                nc.scalar.activation(out=e, in_=sc, func=AF.Exp,
                                                     scale=1.0 / 16.0)
                                nc.tensor.matmul(sums, ones_sm, e,
                                                 start=(kt == 0),
                                                 stop=(kt == KC - 1))
                                nc.tensor.matmul(ctx0, vh[:, kt, 0:128], e,
                                                 start=(kt == 0),
                                                 stop=(kt == KC - 1))
                                nc.tensor.matmul(ctx1, vh[:, kt, 128:256], e,
                                                 start=(kt == 0),
                                                 stop=(kt == KC - 1))
                            rec = pc_o.tile([128, 512], F32, tag="rec",
                                            name="rec")
                            nc.vector.reciprocal(out=rec, in_=sums)
                            for j, ctx in enumerate((ctx0, ctx1)):
                                tmp = pc_o.tile([128, 512], F32, tag="ctmp",
                                                name="ctmp")
                                nc.vector.tensor_mul(tmp, ctx, rec)
                                nc.scalar.activation(
                                    out=ctxt_sb[:, h * 2 + j, qsl], in_=tmp,
                                    func=AF.Identity,
                                    bias=bv_sb[:, h * 2 + j:h * 2 + j + 1])

            # =========== Phase D: gated integration MLP ===========
            # cat == [x ; ctx]  (out_w folded into gate/int weights on host)
            def cat_chunk(hc):
                return xt_sb[:, hc, :] if hc < HC else ctxt_sb[:, hc - HC, :]

            pd_w2 = sbc.enter_context(tc.tile_pool(name="pd_w2", bufs=2))
            l2g_bc = singles.tile([128, H], F32)
            nc.sync.dma_start(
                out=l2g_bc,
                in_=d_l2g[:].unsqueeze(0).partition_broadcast(128).squeeze(1))
            l2b_bc = singles.tile([128, H], F32)
            nc.sync.dma_start(
                out=l2b_bc,
                in_=d_l2b[:].unsqueeze(0).partition_broadcast(128).squeeze(1))
            with ExitStack() as sd:
                pd = sd.enter_context(tc.tile_pool(name="pd", bufs=1))
                h1_sb = pd.tile([128, H2C, R], F32R)   # 8 MB
                with ExitStack() as sd12:
                    pd_st = sd12.enter_context(tc.tile_pool(name="pd_st",
                                                            bufs=1))
                    mu2_sb = pd_st.tile([128, R], F32)
                    rstd2_sb = pd_st.tile([128, R], F32)
                    pd_w1 = sd12.enter_context(tc.tile_pool(name="pd_w1",
                                                            bufs=3))
                    pd_sq = sd12.enter_context(tc.tile_pool(name="pd_sq",
                                                            bufs=2))
                    pd_ps = sd12.enter_context(
                        tc.tile_pool(name="pd_ps", bufs=1, space="PSUM"))
                    h1ps = [pd_ps.tile([128, 512], F32, tag=f"h1ps{i}",
                                       name=f"h1ps{i}") for i in range(4)]
                    for oc2 in range(H2C):
                        w1s = pd_w1.tile([128, H2C, 128], F32R, tag="w1",
                                         name="w1s")
                        nc.sync.dma_start(out=w1s, in_=d_w1[oc2])
                        for rt in range(RT):
                            sl = bass.ts(rt, 512)
                            ps = h1ps[(oc2 * RT + rt) % 4]
                            for hc in range(H2C):
                                nc.tensor.matmul(ps, w1s[:, hc, :],
                                                 cat_chunk(hc)[:, sl],
                                                 start=(hc == 0),
                                                 stop=(hc == H2C - 1))
                            nc.scalar.activation(out=h1_sb[:, oc2, sl], in_=ps,
                                                 func=AF.Identity,
                                                 bias=b1_sb[:, oc2:oc2 + 1])
                    # D2: layernorm over 2048 features + exact gelu (in place)
                    mu2_ps = [pd_ps.tile([128, 512], F32, tag=f"m2_{i}",
                                         name=f"m2_{i}") for i in range(RT)]
                    ms2_ps = [pd_ps.tile([128, 512], F32, tag=f"s2_{i}",
                                         name=f"s2_{i}") for i in range(RT)]
                    for oc2 in range(H2C):
                        sq = pd_sq.tile([128, R], F32R, tag="sqt1", name="sq2")
                        nc.vector.tensor_mul(sq, h1_sb[:, oc2, :].bitcast(F32),
                                             h1_sb[:, oc2, :].bitcast(F32))
                        for i in range(RT):
                            sl = bass.ts(i, 512)
                            nc.tensor.matmul(mu2_ps[i], ones_2k,
                                             h1_sb[:, oc2, sl],
                                             start=(oc2 == 0),
                                             stop=(oc2 == H2C - 1))
                            nc.tensor.matmul(ms2_ps[i], ones_2k, sq[:, sl],
                                             start=(oc2 == 0),
                                             stop=(oc2 == H2C - 1))
                    for i in range(RT):
                        sl = bass.ts(i, 512)
                        nc.scalar.activation(out=mu2_sb[:, sl], in_=mu2_ps[i],
                                             func=AF.Copy)
                        var = pd_sq.tile([128, 512], F32, tag="var2",
                                         name="var2")
                        nc.vector.tensor_mul(var, mu2_sb[:, sl], mu2_sb[:, sl])
                        nc.vector.tensor_sub(var, ms2_ps[i], var)
                        nc.scalar.activation(out=var, in_=var, func=AF.Ln,
                                             bias=eps_t, scale=1.0)
                        nc.scalar.activation(out=rstd2_sb[:, sl], in_=var,
                                             func=AF.Exp, scale=-0.5)
                    for oc2 in range(H2C):
                        t1 = pd_sq.tile([128, R], F32, tag="sqt1", name="t1d")
                        nc.vector.tensor_sub(t1, h1_sb[:, oc2, :].bitcast(F32),
                                             mu2_sb)
                        nc.vector.scalar_tensor_tensor(
                            out=t1, in0=t1, scalar=ilg_sb[:, oc2:oc2 + 1],
                            in1=rstd2_sb, op0=OP.mult, op1=OP.mult)
                        nc.scalar.activation(out=h1_sb[:, oc2, :], in_=t1,
                                             func=AF.Gelu,
                                             bias=ilb_sb[:, oc2:oc2 + 1])
                # D3: integ = gelu(h1) @ w2.T + b2; gate = sigmoid(cat@gw.T+gb)
                #     y = x + gate * integ         (feature-major, fp32)
                with ExitStack() as sd34:
                    pd_y = sd34.enter_context(tc.tile_pool(name="pd_y", bufs=1))
                    yt_sb = pd_y.tile([128, HC, R], F32)
                    pd_o = sd34.enter_context(tc.tile_pool(name="pd_o", bufs=2))
                    pd_yr = sd34.enter_context(tc.tile_pool(name="pd_yr",
                                                            bufs=2))
                    pd_ps3 = sd34.enter_context(
                        tc.tile_pool(name="pd_ps3", bufs=2, space="PSUM"))
                    pd_ps4 = sd34.enter_context(
                        tc.tile_pool(name="pd_ps4", bufs=2, space="PSUM"))

                    def d4_chunk(rc):
                        tp = pd_ps4.tile([128, 1024], F32, tag="tp", name="tp")
                        for oc in range(HC):
                            nc.tensor.transpose(
                                tp[:, oc * 128:(oc + 1) * 128],
                                yt_sb[:, oc, rc * 128:(rc + 1) * 128], ident)
                        yr = pd_yr.tile([128, H], F32, tag="yr", name="yr")
                        nc.scalar.activation(out=yr[:, 0:512], in_=tp[:, 0:512],
                                             func=AF.Copy)
                        nc.scalar.activation(out=yr[:, 512:1024],
                                             in_=tp[:, 512:1024], func=AF.Copy)
                        stats = pd_o.tile([128, 2, 6], F32, tag="bst",
                                          name="bst")
                        for i in range(2):
                            nc.vector.bn_stats(out=stats[:, i, :],
                                               in_=yr[:, i * 512:(i + 1) * 512])
                        mv = pd_o.tile([128, 2], F32, tag="mv", name="mv")
                        nc.vector.bn_aggr(out=mv, in_=stats)
                        sd_ = pd_o.tile([128, 1], F32, tag="sd", name="sd")
                        nc.scalar.activation(out=sd_, in_=mv[:, 1:2],
                                             func=AF.Sqrt, bias=eps_t, scale=1.0)
                        rstd = pd_o.tile([128, 1], F32, tag="rsd", name="rstd")
                        nc.vector.reciprocal(out=rstd, in_=sd_)
                        nmr = pd_o.tile([128, 1], F32, tag="nmr", name="nmr")
                        nc.vector.scalar_tensor_tensor(
                            out=nmr, in0=mv[:, 0:1], scalar=-1.0, in1=rstd,
                            op0=OP.mult, op1=OP.mult)
                        nc.scalar.activation(out=yr, in_=yr, func=AF.Identity,
                                             bias=nmr, scale=rstd)
                        nc.vector.tensor_mul(yr, yr, l2g_bc)
                        nc.vector.tensor_add(yr, yr, l2b_bc)
                        nc.sync.dma_start(out=d_out[rc * 128:(rc + 1) * 128, :],
                                          in_=yr)

                    for rt in range(RT):
                        sl = bass.ts(rt, 512)
                        for oc in range(HC):
                            w2s = pd_w2.tile([128, H2C, 128], F32R, tag="w23",
                                             name="w2s")
                            gws = pd_w2.tile([128, H2C, 128], F32R, tag="w23",
                                             name="gws")
                            nc.sync.dma_start(out=gws, in_=d_gw[oc])
                            nc.sync.dma_start(out=w2s, in_=d_w2[oc])
                            gps = pd_ps3.tile([128, 512], F32, tag="gps",
                                              name="gps")
                            for hc in range(H2C):
                                nc.tensor.matmul(gps, gws[:, hc, :],
                                                 cat_chunk(hc)[:, sl],
                                                 start=(hc == 0),
                                                 stop=(hc == H2C - 1))
                            igps = pd_ps3.tile([128, 512], F32, tag="igps",
                                               name="igps")
                            for hc in range(H2C):
                                nc.tensor.matmul(igps, w2s[:, hc, :],
                                                 h1_sb[:, hc, sl],
                                                 start=(hc == 0),
                                                 stop=(hc == H2C - 1))
                            sig = pd_o.tile([128, 512], F32, tag="sig",
                                            name="sig", bufs=4)
                            nc.scalar.activation(out=sig, in_=gps,
                                                 func=AF.Sigmoid,
                                                 bias=gb_sb[:, oc:oc + 1])
                            tmp = pd_o.tile([128, 512], F32, tag="ytmp",
                                            name="ytmp")
                            nc.vector.scalar_tensor_tensor(
                                out=tmp, in0=igps, scalar=b2_sb[:, oc:oc + 1],
                                in1=sig, op0=OP.add, op1=OP.mult)
                            nc.vector.tensor_add(yt_sb[:, oc, sl], tmp,
                                                 xt_sb[:, oc, sl].bitcast(F32))
                        for rc in range(rt * 4, rt * 4 + 4):
                            d4_chunk(rc)

    nc.compile()
    return nc


_NC_CACHE = []


def _get_nc():
    if not _NC_CACHE:
        _NC_CACHE.append(build_program())
    return _NC_CACHE[0]


def kernel(query_hidden, mem_keys, importance, recency, access_count,
           Wq, bq, in_w, in_b, out_w, out_b, gate_w, gate_b,
           int_w1, int_b1, int_ln_g, int_ln_b, int_w2, int_b2,
           ln1_g, ln1_b, ln2_g, ln2_b, sel_params, top_k):
    np32 = lambda a: np.asarray(a, dtype=np.float32)
    query_hidden = np32(query_hidden)
    mem_keys = np32(mem_keys)
    top_k = int(top_k)
    assert top_k == K, f"kernel compiled for top_k={K}, got {top_k}"

    # HTPS selection (host): softmax-weighted score, top-k set, gather.
    # Attention output is invariant to the order of the selected rows, so an
    # argpartition set (== jax.lax.top_k set) is sufficient.
    sp = np32(sel_params)
    w = np.exp(sp - sp.max())
    w = w / w.sum()
    acc = np32(access_count)
    sel = w[0] * np32(importance) + w[1] * np32(recency) + w[2] * (acc / acc.max())
    idx = np.argpartition(-sel, top_k - 1)[:top_k]
    mem_t = np.ascontiguousarray(mem_keys[idx].T)      # [H, K]

    in_w = np32(in_w)
    in_b = np32(in_b)
    wq, wk, wv = in_w[:H], in_w[H:2 * H], in_w[2 * H:]
    bqi, bki, bvi = in_b[:H], in_b[H:2 * H], in_b[2 * H:]
    wc = wq @ np32(Wq)                                  # fused query projection
    bc = wq @ np32(bq) + bqi

    # fold mem-layernorm gamma/beta into the K/V projections
    g1 = np32(ln1_g)
    b1v = np32(ln1_b)
    bki = bki + wk @ b1v
    bvi = bvi + wv @ b1v
    wk = wk * g1[None, :]
    wv = wv * g1[None, :]

    # fold attn_out = ctx @ out_w.T + out_b into the gate / integration weights
    out_w = np32(out_w)
    out_b = np32(out_b)
    gate_w = np32(gate_w)
    int_w1 = np32(int_w1)
    gwx, gwa = gate_w[:, :H], gate_w[:, H:]
    w1x, w1a = int_w1[:, :H], int_w1[:, H:]
    gate_b_f = np32(gate_b) + gwa @ out_b
    int_b1_f = np32(int_b1) + w1a @ out_b

    T = lambda a: np.ascontiguousarray(np32(a).T)

    def chunked(w_t, ow=128):
        # [IN, OUT] -> [OUT//ow, 128, IN//128, ow]: contiguous per-partition slabs
        inn, out = w_t.shape
        r = w_t.reshape(inn // 128, 128, out // ow, ow).transpose(2, 1, 0, 3)
        return np.ascontiguousarray(r)

    gw_t = np.concatenate([gwx.T, (gwa @ out_w).T], axis=0)
    w1_t = np.concatenate([w1x.T, (w1a @ out_w).T], axis=0)

    common = {
        "mem_t": mem_t,
        "wc_t": chunked(T(wc)), "wk_t": chunked(T(wk)),
        "wv_t": chunked(T(wv), ow=512),
        "gw_t": chunked(gw_t), "w1_t": chunked(w1_t),
        "w2_t": chunked(T(int_w2)),
        "bc": bc, "bk": bki, "bv": bvi,
        "gate_b": gate_b_f, "int_b1": int_b1_f, "int_b2": np32(int_b2),
        "iln_g": np32(int_ln_g), "iln_b": np32(int_ln_b),
        "ln2_g": np32(ln2_g), "ln2_b": np32(ln2_b),
    }
    X = query_hidden.reshape(B * S, H)
    in_maps = []
    for c in range(N_CORES):
        m = dict(common)
        m["x_t"] = np.ascontiguousarray(X[c * R:(c + 1) * R].T)
        in_maps.append(m)

    nc = _get_nc()
    res = run_bass_kernel_spmd(nc, in_maps, core_ids=list(range(N_CORES)))
    out = np.empty((B * S, H), dtype=np.float32)
    for c in range(N_CORES):
        out[c * R:(c + 1) * R] = res.results[c]["out"]
    return out.reshape(B, S, H)



# revision 11
# speedup vs baseline: 1.6649x; 1.6649x over previous
"""MemoryRetriever kernel for 8x Trainium2 NeuronCores — fp8 DoubleRow edition.

Data-parallel over the B*S=8192 query rows (1024 rows/core); the selected
memory bank and all weights are replicated.

Precision plan (validated against the reference on host):
  - attention block (K/V/Q projections, scores, softmax, ctx) and the gate
    run in fp8e4m3 DoubleRow matmuls (2.07x fp32r throughput measured);
  - the x-side of the integration MLP (h1x) runs in fp32r, the ctx side in
    fp8 (ctx is tiny so its quantization error is negligible);
  - integ + LN stats run in bf16; final residual/LN in fp32.

fp8 weights are pre-scaled on host (x32 / x1024) to dodge e4m3 subnormals;
descales are folded into per-partition scalars downstream.

Linear-algebra folds (host, exact):
  - memory layernorm is applied as a rank-1 correction AFTER the K/V
    projections: K = rstd.(Wk'@mem) - outer(rowsum(Wk'), mu.rstd); the
    K-side bias cancels in softmax entirely; the V-side bias is folded into
    the gate/integration biases (sum(attn)=1).
  - query_proj+Wq fused; out_w folded into gate/integration weights.
"""

import sys
from contextlib import ExitStack

if "/opt/trn_rl_repo" not in sys.path:
    sys.path.insert(0, "/opt/trn_rl_repo")

import numpy as np
import ml_dtypes

import concourse.bass as bass
import concourse.mybir as mybir
import concourse.tile as tile
from concourse import bacc
from concourse.bass_utils import run_bass_kernel_spmd
from concourse.masks import make_identity

F32 = mybir.dt.float32
F32R = mybir.dt.float32r
BF16 = mybir.dt.bfloat16
FP8 = mybir.dt.float8e4
AF = mybir.ActivationFunctionType
OP = mybir.AluOpType
DR = mybir.MatmulPerfMode.DoubleRow

H = 1024
NH = 4
HD = H // NH          # 256
K = 2048              # top_k
B, S = 4, 2048
N_CORES = 8
R = (B * S) // N_CORES  # 1024 rows per core
EPS = 1e-5
H2 = 2 * H

HC = H // 128         # 8
H2C = H2 // 128       # 16
KC = K // 128         # 16 key chunks
KCP = KC // 2         # 8 key-chunk pairs
RT = R // 512         # 2 row tiles of 512
RT4 = R // 256        # 4 row tiles of 256

SQ = 32.0             # wc fp8 pre-scale
SKV = 32.0            # wk/wv fp8 pre-scale
SGX = 1024.0          # gwx fp8 pre-scale
SGA = 32.0            # gwa' fp8 pre-scale (x32 from ctx8 -> 1024 total)
S1A = 32.0            # w1a' fp8 pre-scale

NP_FP8 = ml_dtypes.float8_e4m3
NP_BF16 = ml_dtypes.bfloat16


def build_program(ln2_trivial):
    nc = bacc.Bacc("TRN2", target_bir_lowering=False)

    dram = {}

    def dp(name, shape, dt):
        dram[name] = nc.declare_dram_parameter(name, shape, dt, isOutput=False)
        return dram[name]

    d_x8 = dp("x8", [128, HC, R], FP8)
    d_xt = dp("xt", [128, HC, R], F32R)
    d_m8 = dp("m8", [128, HC + 2, K], FP8)
    d_wk8 = dp("wk8", [128, HC, 5, 2, 128], FP8)
    d_wc8 = dp("wc8", [128, HC, 4, 2, 128], FP8)
    d_wv8 = dp("wv8", [128, 5, 2, H], FP8)
    d_gwx8 = dp("gwx8", [128, HC, 4, 2, 128], FP8)
    d_gwa8 = dp("gwa8", [128, HC, 4, 2, 128], FP8)
    d_w1a8 = dp("w1a8", [128, H2C, 4, 2, 128], FP8)
    d_w1x = dp("w1x", [H2C, 128, HC, 128], F32R)
    d_w2b = dp("w2b", [128, HC, H2C, 128], BF16)
    d_cv = dp("cv", [K], F32)       # rstd/SKV
    d_bc = dp("bc", [H], F32)       # fused q bias
    d_gb = dp("gb", [H], F32)
    d_b1 = dp("b1", [H2], F32)
    d_b2 = dp("b2", [H], F32)
    d_ilg = dp("ilg", [H2], F32)
    d_ilb = dp("ilb", [H2], F32)
    d_l2g = dp("l2g", [H], F32)
    d_l2b = dp("l2b", [H], F32)
    d_out = nc.declare_dram_parameter("out", [R, H], F32, isOutput=True)

    def bcast(pool, vec, n, nm):
        t = pool.tile([128, n], F32, tag=f"bc_{nm}", name=f"bc_{nm}")
        nc.sync.dma_start(
            out=t, in_=vec[:].unsqueeze(0).partition_broadcast(128).squeeze(1))
        return t

    def load_pp(pool, vec, n, nm):
        t = pool.tile([128, n], F32, tag=f"pp_{nm}", name=f"pp_{nm}")
        nc.sync.dma_start(out=t, in_=vec[:].rearrange("(c p) -> p c", p=128))
        return t

    with tile.TileContext(nc) as tc, ExitStack() as top:
        singles = top.enter_context(tc.tile_pool(name="singles", bufs=1))

        ident = singles.tile([128, 128], F32)
        make_identity(nc, ident)
        scr1 = singles.tile([128, 128], F32)
        nc.vector.memset(scr1, 1.0)
        ones8 = singles.tile([128, 2, 128], FP8)
        nc.scalar.activation(out=ones8[:, 0, :], in_=scr1, func=AF.Copy)
        nc.scalar.activation(out=ones8[:, 1, :], in_=scr1, func=AF.Copy)
        ones2k = singles.tile([128, 128], BF16)
        nc.scalar.activation(out=ones2k, in_=scr1, func=AF.Copy,
                             scale=1.0 / 2048.0)
        eps_t = singles.tile([128, 1], F32)
        nc.vector.memset(eps_t, EPS)

        bc_pp = load_pp(singles, d_bc, HC, "bc")
        gb_pp = load_pp(singles, d_gb, HC, "gb")
        b1_pp = load_pp(singles, d_b1, H2C, "b1")
        b2_pp = load_pp(singles, d_b2, HC, "b2")
        ilg_pp = load_pp(singles, d_ilg, H2C, "ilg")
        ilb_pp = load_pp(singles, d_ilb, H2C, "ilb")

        # long-lived activations
        x8 = singles.tile([128, HC, R], FP8)
        nc.sync.dma_start(out=x8, in_=d_x8[:, :, :])
        xt = singles.tile([128, HC, R], F32R)
        nc.sync.dma_start(out=xt, in_=d_xt[:, :, :])
        K8 = singles.tile([128, NH, 2, K], FP8)
        V8 = singles.tile([128, KCP, 2, H], FP8)
        ctx8 = singles.tile([128, HC, R], FP8)
        h1b = singles.tile([128, H2C, R], BF16)

        # =============== Phase A: K/V projections over the bank ============
        with ExitStack() as sA:
            pa = sA.enter_context(tc.tile_pool(name="pa", bufs=1))
            pa_ps = sA.enter_context(
                tc.tile_pool(name="pa_ps", bufs=1, space="PSUM"))
            mem8 = pa.tile([128, HC + 2, K], FP8)
            nc.sync.dma_start(out=mem8, in_=d_m8[:, :, :])
            wk8 = pa.tile([128, HC, 5, 2, 128], FP8)
            nc.sync.dma_start(out=wk8, in_=d_wk8[:, :, :, :, :])
            wv8 = pa.tile([128, 5, 2, H], FP8)
            nc.sync.dma_start(out=wv8, in_=d_wv8[:, :, :, :])
            c_bc = bcast(pa, d_cv, K, "cv")
            cv_pp = load_pp(pa, d_cv, KC, "cvp")

            # LN1 rank-1 corrections ride as a 5th contraction pair:
            # mem8[:, 8:10] row (0,0) holds 32*mu; wk8[...,4] and wv8[:, 4]
            # hold the negated weight rowsums.
            for kt in range(4):
                ksl = bass.ts(kt, 512)
                for oc in range(HC):
                    ps = pa_ps.tile([128, 512], F32, tag="psK", name="psK",
                                    bufs=3)
                    for i in range(5):
                        nc.tensor.matmul(ps, wk8[:, oc, i],
                                         mem8[:, 2 * i:2 * i + 2, ksl],
                                         start=(i == 0), stop=(i == 4),
                                         perf_mode=DR)
                    nc.vector.tensor_mul(K8[:, oc // 2, oc % 2, ksl], ps,
                                         c_bc[:, ksl])
                for kc in range(kt * 4, kt * 4 + 4):
                    for fh in range(2):
                        fsl = bass.ts(fh, 512)
                        ps = pa_ps.tile([128, 512], F32, tag="psV",
                                        name="psV", bufs=3)
                        for i in range(5):
                            nc.tensor.matmul(
                                ps, mem8[:, 2 * i:2 * i + 2,
                                         kc * 128:(kc + 1) * 128],
                                wv8[:, i, :, fsl],
                                start=(i == 0), stop=(i == 4), perf_mode=DR)
                        nc.scalar.activation(out=V8[:, kc // 2, kc % 2, fsl],
                                             in_=ps, func=AF.Copy,
                                             scale=cv_pp[:, kc:kc + 1])

        # =============== Phases B+C: query proj + attention =================
        with ExitStack() as sBC:
            pb = sBC.enter_context(tc.tile_pool(name="pb", bufs=1))
            pb_t = sBC.enter_context(tc.tile_pool(name="pb_t", bufs=4))
            pb_e = sBC.enter_context(tc.tile_pool(name="pb_e", bufs=4))
            pb_ps = sBC.enter_context(
                tc.tile_pool(name="pb_ps", bufs=1, space="PSUM"))
            pw1 = sBC.enter_context(tc.tile_pool(name="pw1", bufs=4))

            wc8 = pb.tile([128, HC, 4, 2, 128], FP8)
            nc.sync.dma_start(out=wc8, in_=d_wc8[:, :, :, :, :])
            w1a8 = pb.tile([128, H2C, 4, 2, 128], FP8)
            nc.sync.dma_start(out=w1a8, in_=d_w1a8[:, :, :, :, :])
            Q8 = pb.tile([128, HC, R], FP8)

            def q_group(h, oc, rt):
                rsl = bass.ts(rt, 512)
                ps = pb_ps.tile([128, 512], F32, tag="fill", name="fill",
                                bufs=2)
                for i in range(4):
                    nc.tensor.matmul(ps, wc8[:, oc, i],
                                     x8[:, 2 * i:2 * i + 2, rsl],
                                     start=(i == 0), stop=(i == 3),
                                     perf_mode=DR)
                nc.vector.tensor_scalar(
                    out=Q8[:, oc, rsl], in0=ps, scalar1=1.0 / SQ,
                    scalar2=bc_pp[:, oc:oc + 1], op0=OP.mult, op1=OP.add)

            w1slab = {}

            def h1x_group(oc2, rt):
                rsl = bass.ts(rt, 512)
                if oc2 not in w1slab:
                    sl = pw1.tile([128, HC, 128], F32R, tag="w1x",
                                  name=f"w1x{oc2}")
                    nc.sync.dma_start(out=sl, in_=d_w1x[oc2])
                    w1slab[oc2] = sl
                ps = pb_ps.tile([128, 512], F32, tag="fill", name="fill",
                                bufs=2)
                for hc in range(HC):
                    nc.tensor.matmul(ps, w1slab[oc2][:, hc], xt[:, hc, rsl],
                                     start=(hc == 0), stop=(hc == HC - 1))
                nc.vector.tensor_scalar(
                    out=h1b[:, oc2, rsl], in0=ps,
                    scalar1=b1_pp[:, oc2:oc2 + 1], scalar2=None, op0=OP.add)

            # fillers: Q heads 1..3 first, then leading h1x groups
            fillers = [(q_group, (h, 2 * h + j, rt))
                       for h in range(1, NH) for j in range(2)
                       for rt in range(RT)]
            fillers += [(h1x_group, (oc2, rt))
                        for oc2 in range(H2C) for rt in range(RT)]
            fillers.reverse()  # pop from the end
            emitted_h1x = set()

            def pop_filler():
                if fillers:
                    fn, args = fillers.pop()
                    fn(*args)
                    if fn is h1x_group:
                        emitted_h1x.add(args)

            # Q for head 0 up front
            for j in range(2):
                for rt in range(RT):
                    q_group(0, j, rt)

            for h in range(NH):
                for rt in range(RT):
                    rsl = bass.ts(rt, 512)
                    sums = pb_ps.tile([128, 512], F32, tag="sums",
                                      name="sums", bufs=2)
                    cps = [pb_ps.tile([128, 512], F32, tag=f"ctx{fg}",
                                      name=f"ctx{fg}", bufs=1)
                           for fg in range(2)]
                    for p in range(KCP):
                        e8p = pb_e.tile([128, 2, 512], FP8, tag="e8",
                                        name="e8")
                        for j in range(2):
                            kc = 2 * p + j
                            sc = pb_ps.tile([128, 512], F32, tag="sc",
                                            name="sc", bufs=2)
                            nc.tensor.matmul(
                                sc, K8[:, h, :, kc * 128:(kc + 1) * 128],
                                Q8[:, 2 * h:2 * h + 2, rsl],
                                start=True, stop=True, perf_mode=DR)
                            nc.scalar.activation(out=e8p[:, j, :], in_=sc,
                                                 func=AF.Exp, scale=1.0 / 16)
                        if p % 2 == 1:
                            pop_filler()
                        nc.tensor.matmul(sums, ones8, e8p,
                                         start=(p == 0), stop=(p == KCP - 1),
                                         perf_mode=DR)
                        for fg in range(2):
                            nc.tensor.matmul(
                                cps[fg],
                                V8[:, p, :,
                                   h * HD + fg * 128:h * HD + (fg + 1) * 128],
                                e8p, start=(p == 0), stop=(p == KCP - 1),
                                perf_mode=DR)
                    rec = pb_t.tile([128, 512], F32, tag="rec", name="rec")
                    nc.vector.reciprocal_approx_fast(out=rec, in_=sums)
                    for fg in range(2):
                        nc.vector.scalar_tensor_tensor(
                            out=ctx8[:, 2 * h + fg, rsl], in0=cps[fg],
                            scalar=32.0, in1=rec, op0=OP.mult, op1=OP.mult)

            # =============== Phase D1: finish h1 ===========================
            for oc2 in range(H2C):
                for rt in range(RT):
                    if (oc2, rt) not in emitted_h1x:
                        h1x_group(oc2, rt)
                    rsl = bass.ts(rt, 512)
                    ps2 = pb_ps.tile([128, 512], F32, tag="fill", name="psH",
                                     bufs=2)
                    for i in range(4):
                        nc.tensor.matmul(ps2, w1a8[:, oc2, i],
                                         ctx8[:, 2 * i:2 * i + 2, rsl],
                                         start=(i == 0), stop=(i == 3),
                                         perf_mode=DR)
                    nc.vector.scalar_tensor_tensor(
                        out=h1b[:, oc2, rsl], in0=ps2, scalar=1.0 / 1024.0,
                        in1=h1b[:, oc2, rsl], op0=OP.mult, op1=OP.add)

        # =============== Phase D2: iLN stats + apply + gelu ================
        with ExitStack() as sD2:
            pd2 = sD2.enter_context(tc.tile_pool(name="pd2", bufs=1))
            pd2_t = sD2.enter_context(tc.tile_pool(name="pd2_t", bufs=3))
            pd2_ps = sD2.enter_context(
                tc.tile_pool(name="pd2_ps", bufs=1, space="PSUM"))
            mu2_sb = pd2.tile([128, R], F32)
            rstd2_sb = pd2.tile([128, R], F32)
            mu2_ps = [pd2_ps.tile([128, 512], F32, tag=f"m2_{i}",
                                  name=f"m2_{i}") for i in range(RT)]
            ms2_ps = [pd2_ps.tile([128, 512], F32, tag=f"s2_{i}",
                                  name=f"s2_{i}") for i in range(RT)]
            for oc2 in range(H2C):
                sq = pd2_t.tile([128, R], BF16, tag="sq", name="sq")
                eng = nc.vector if oc2 % 2 == 0 else nc.gpsimd
                eng.tensor_mul(sq, h1b[:, oc2, :], h1b[:, oc2, :])
                for rt in range(RT):
                    rsl = bass.ts(rt, 512)
                    nc.tensor.matmul(mu2_ps[rt], ones2k, h1b[:, oc2, rsl],
                                     start=(oc2 == 0), stop=(oc2 == H2C - 1))
                    nc.tensor.matmul(ms2_ps[rt], ones2k, sq[:, rsl],
                                     start=(oc2 == 0), stop=(oc2 == H2C - 1))
            for rt in range(RT):
                rsl = bass.ts(rt, 512)
                nc.scalar.activation(out=mu2_sb[:, rsl], in_=mu2_ps[rt],
                                     func=AF.Copy)
                var = pd2_t.tile([128, 512], F32, tag="var", name="var")
                nc.vector.tensor_mul(var, mu2_sb[:, rsl], mu2_sb[:, rsl])
                nc.vector.tensor_sub(var, ms2_ps[rt], var)
                nc.scalar.activation(out=var, in_=var, func=AF.Ln,
                                     bias=eps_t, scale=1.0)
                nc.scalar.activation(out=rstd2_sb[:, rsl], in_=var,
                                     func=AF.Exp, scale=-0.5)
            for oc2 in range(H2C):
                td = pd2_t.tile([128, R], F32, tag="td", name="td")
                eng = nc.vector if oc2 % 2 == 0 else nc.gpsimd
                eng.tensor_sub(td, h1b[:, oc2, :], mu2_sb)
                nc.vector.scalar_tensor_tensor(
                    out=td, in0=td, scalar=ilg_pp[:, oc2:oc2 + 1],
                    in1=rstd2_sb, op0=OP.mult, op1=OP.mult)
                nc.scalar.activation(out=h1b[:, oc2, :], in_=td, func=AF.Gelu,
                                     bias=ilb_pp[:, oc2:oc2 + 1])

        # =============== Phase D3+D4: gate, integ, residual, LN2 ===========
        with ExitStack() as sD3:
            pd3 = sD3.enter_context(tc.tile_pool(name="pd3", bufs=1))
            pd3_t = sD3.enter_context(tc.tile_pool(name="pd3_t", bufs=4))
            pd3_y = sD3.enter_context(tc.tile_pool(name="pd3_y", bufs=2))
            pd3_o = sD3.enter_context(tc.tile_pool(name="pd3_o", bufs=3))
            pd3_ps = sD3.enter_context(
                tc.tile_pool(name="pd3_ps", bufs=1, space="PSUM"))
            pd3_pt = sD3.enter_context(
                tc.tile_pool(name="pd3_pt", bufs=2, space="PSUM"))
            w2b = pd3.tile([128, HC, H2C, 128], BF16)
            nc.sync.dma_start(out=w2b, in_=d_w2b[:, :, :, :])
            gwx8 = pd3.tile([128, HC, 4, 2, 128], FP8)
            nc.sync.dma_start(out=gwx8, in_=d_gwx8[:, :, :, :, :])
            gwa8 = pd3.tile([128, HC, 4, 2, 128], FP8)
            nc.sync.dma_start(out=gwa8, in_=d_gwa8[:, :, :, :, :])
            if not ln2_trivial:
                l2g_bc = bcast(pd3, d_l2g, H, "l2g")
                l2b_bc = bcast(pd3, d_l2b, H, "l2b")

            def d4_chunk(yt, rt4, rc2):
                row0 = rt4 * 256 + rc2 * 128
                tp = pd3_pt.tile([128, H], F32, tag="tp", name="tp")
                for oc in range(HC):
                    nc.tensor.transpose(
                        tp[:, oc * 128:(oc + 1) * 128],
                        yt[:, oc, rc2 * 128:(rc2 + 1) * 128], ident)
                bst = pd3_o.tile([128, 2, 6], F32, tag="bst", name="bst")
                for i in range(2):
                    nc.vector.bn_stats(out=bst[:, i, :],
                                       in_=tp[:, i * 512:(i + 1) * 512])
                mv = pd3_o.tile([128, 2], F32, tag="mv", name="mv")
                nc.vector.bn_aggr(out=mv, in_=bst)
                sd = pd3_o.tile([128, 1], F32, tag="sd", name="sd")
                nc.scalar.activation(out=sd, in_=mv[:, 1:2], func=AF.Sqrt,
                                     bias=eps_t, scale=1.0)
                rstd = pd3_o.tile([128, 1], F32, tag="rsd", name="rsd")
                nc.vector.reciprocal(out=rstd, in_=sd)
                nmr = pd3_o.tile([128, 1], F32, tag="nmr", name="nmr")
                nc.vector.scalar_tensor_tensor(
                    out=nmr, in0=mv[:, 0:1], scalar=-1.0, in1=rstd,
                    op0=OP.mult, op1=OP.mult)
                yr = pd3_o.tile([128, H], F32, tag="yr", name="yr")
                for i in range(2):
                    nc.scalar.activation(out=yr[:, i * 512:(i + 1) * 512],
                                         in_=tp[:, i * 512:(i + 1) * 512],
                                         func=AF.Identity, bias=nmr,
                                         scale=rstd)
                if not ln2_trivial:
                    nc.vector.tensor_mul(yr, yr, l2g_bc)
                    nc.vector.tensor_add(yr, yr, l2b_bc)
                nc.sync.dma_start(out=d_out[row0:row0 + 128, :], in_=yr)

            for rt4 in range(RT4):
                r0 = rt4 * 256
                rsl = slice(r0, r0 + 256)
                yt = pd3_y.tile([128, HC, 256], F32, tag="yt", name="yt")
                for oc in range(HC):
                    gps = pd3_ps.tile([128, 256], F32, tag="gps", name="gps",
                                      bufs=2)
                    for i in range(4):
                        nc.tensor.matmul(gps, gwx8[:, oc, i],
                                         x8[:, 2 * i:2 * i + 2, rsl],
                                         start=(i == 0), stop=False,
                                         perf_mode=DR)
                    for i in range(4):
                        nc.tensor.matmul(gps, gwa8[:, oc, i],
                                         ctx8[:, 2 * i:2 * i + 2, rsl],
                                         start=False, stop=(i == 3),
                                         perf_mode=DR)
                    sig = pd3_t.tile([128, 256], F32, tag="sig", name="sig")
                    nc.scalar.activation(out=sig, in_=gps, func=AF.Sigmoid,
                                         bias=gb_pp[:, oc:oc + 1],
                                         scale=1.0 / SGX)
                    ips = pd3_ps.tile([128, 256], F32, tag="ips", name="ips",
                                      bufs=2)
                    for hc2 in range(H2C):
                        nc.tensor.matmul(ips, w2b[:, oc, hc2],
                                         h1b[:, hc2, rsl],
                                         start=(hc2 == 0),
                                         stop=(hc2 == H2C - 1))
                    tmp = pd3_t.tile([128, 256], F32, tag="ytmp", name="ytmp")
                    nc.vector.scalar_tensor_tensor(
                        out=tmp, in0=ips, scalar=b2_pp[:, oc:oc + 1],
                        in1=sig, op0=OP.add, op1=OP.mult)
                    nc.vector.tensor_add(yt[:, oc, :], tmp,
                                         xt[:, oc, rsl].bitcast(F32))
                for rc2 in range(2):
                    d4_chunk(yt, rt4, rc2)

    nc.compile()
    return nc


_NC_CACHE = {}


def _get_nc(ln2_trivial):
    if ln2_trivial not in _NC_CACHE:
        _NC_CACHE[ln2_trivial] = build_program(ln2_trivial)
    return _NC_CACHE[ln2_trivial]


def kernel(query_hidden, mem_keys, importance, recency, access_count,
           Wq, bq, in_w, in_b, out_w, out_b, gate_w, gate_b,
           int_w1, int_b1, int_ln_g, int_ln_b, int_w2, int_b2,
           ln1_g, ln1_b, ln2_g, ln2_b, sel_params, top_k):
    np32 = lambda a: np.asarray(a, dtype=np.float32)
    f8 = lambda a: np.ascontiguousarray(a.astype(NP_FP8))
    bf = lambda a: np.ascontiguousarray(a.astype(NP_BF16))

    query_hidden = np32(query_hidden)
    mem_keys = np32(mem_keys)
    top_k = int(top_k)
    assert top_k == K, f"kernel compiled for top_k={K}, got {top_k}"

    # HTPS selection (host): softmax-weighted score, top-k set, gather.
    sp = np32(sel_params)
    w = np.exp(sp - sp.max())
    w = w / w.sum()
    acc = np32(access_count)
    sel = (w[0] * np32(importance) + w[1] * np32(recency)
           + w[2] * (acc / acc.max()))
    idx = np.argpartition(-sel, top_k - 1)[:top_k]
    mem = mem_keys[idx]                               # [K, H]

    mu = mem.mean(1)
    rstd = 1.0 / np.sqrt(mem.var(1) + EPS)

    in_w = np32(in_w)
    in_b = np32(in_b)
    wq, wk, wv = in_w[:H], in_w[H:2 * H], in_w[2 * H:]
    bqi, bki, bvi = in_b[:H], in_b[H:2 * H], in_b[2 * H:]
    g1 = np32(ln1_g)
    b1v = np32(ln1_b)

    wkE = wk * g1[None, :]
    wvE = wv * g1[None, :]
    wc = wq @ np32(Wq)
    bcv = wq @ np32(bq) + bqi

    out_w = np32(out_w)
    gate_w = np32(gate_w)
    int_w1 = np32(int_w1)
    gwx, gwa = gate_w[:, :H], gate_w[:, H:]
    w1x, w1a = int_w1[:, :H], int_w1[:, H:]
    bvF = bvi + wv @ b1v                              # V bias (exact)
    out_bF = out_w @ bvF + np32(out_b)
    gate_bF = np32(gate_b) + gwa @ out_bF
    int_b1F = np32(int_b1) + w1a @ out_bF
    gwaE = gwa @ out_w
    w1aE = w1a @ out_w

    def chunk5(wmat, scale):
        # [O, F] -> fp8 [128, O//128, 4, 2, 128] (pair-sliced stationary)
        o, f = wmat.shape
        a = np.ascontiguousarray(wmat.T) * scale      # [F, O]
        a = a.reshape(4, 2, 128, o // 128, 128)       # f = (2i+j)*128 + p
        return f8(np.ascontiguousarray(a.transpose(2, 3, 0, 1, 4)))

    X = query_hidden.reshape(B * S, H)

    def fm(a, dtype_cast):
        # [N, 128*c] -> feature-major [128, c, N]
        n, ftot = a.shape
        t = np.ascontiguousarray(a.T).reshape(ftot // 128, 128, n)
        return dtype_cast(np.ascontiguousarray(t.transpose(1, 0, 2)))

    # extended mem: chunks 8-9 are the LN-correction pair, row (0,0)=32*mu
    m8e = np.zeros((128, HC + 2, K), dtype=NP_FP8)
    m8e[:, :HC, :] = fm(mem, f8)
    m8e[0, HC, :] = (SKV * mu).astype(NP_FP8)

    wk8e = np.zeros((128, HC, 5, 2, 128), dtype=NP_FP8)
    wk8e[:, :, :4] = chunk5(wkE, SKV)
    wk8e[0, :, 4, 0, :] = (-wkE.sum(1)).astype(NP_FP8).reshape(HC, 128)

    wv8e = np.zeros((128, 5, 2, H), dtype=NP_FP8)
    wv8e[:, :4] = f8(np.ascontiguousarray(
        (wvE.T * SKV).reshape(4, 2, 128, H).transpose(2, 0, 1, 3)))
    wv8e[0, 4, 0, :] = (-wvE.sum(1)).astype(NP_FP8)

    common = {
        "m8": m8e,
        "wk8": wk8e,
        "wc8": chunk5(wc, SQ),
        "wv8": wv8e,
        "gwx8": chunk5(gwx, SGX),
        "gwa8": chunk5(gwaE, SGA),
        "w1a8": chunk5(w1aE, S1A),
        "w1x": np.ascontiguousarray(
            w1x.T.reshape(HC, 128, H2C, 128).transpose(2, 1, 0, 3)),
        "w2b": bf(np.ascontiguousarray(
            np32(int_w2).T.reshape(H2C, 128, HC, 128).transpose(1, 2, 0, 3))),
        "cv": rstd / SKV,
        "bc": bcv,
        "gb": gate_bF,
        "b1": int_b1F,
        "b2": np32(int_b2),
        "ilg": np32(int_ln_g),
        "ilb": np32(int_ln_b),
        "l2g": np32(ln2_g),
        "l2b": np32(ln2_b),
    }

    ln2_trivial = bool(np.all(np32(ln2_g) == 1.0)
                       and np.all(np32(ln2_b) == 0.0))

    in_maps = []
    for c in range(N_CORES):
        m = dict(common)
        Xc = X[c * R:(c + 1) * R]
        m["x8"] = fm(Xc, f8)
        m["xt"] = fm(Xc, lambda a: np.ascontiguousarray(a))
        in_maps.append(m)

    nc = _get_nc(ln2_trivial)
    res = run_bass_kernel_spmd(nc, in_maps, core_ids=list(range(N_CORES)))
    out = np.empty((B * S, H), dtype=np.float32)
    for c in range(N_CORES):
        out[c * R:(c + 1) * R] = res.results[c]["out"]
    return out.reshape(B, S, H)


# revision 16
# speedup vs baseline: 1.8876x; 1.1338x over previous
"""MemoryRetriever kernel for 8x Trainium2 NeuronCores — fp8 DoubleRow edition.

Data-parallel over the B*S=8192 query rows (1024 rows/core); the selected
memory bank and all weights are replicated.

Precision plan (validated against the reference on host, maxrel ~1e-2):
  - attention block (K/V/Q projections, scores, softmax, ctx) and the gate
    run in fp8e4m3 DoubleRow matmuls (2.07x fp32r throughput measured);
  - the x-side of the integration MLP (h1x) runs in fp32r, the ctx side in
    fp8 (ctx is tiny so its quantization error is negligible);
  - integ + LN stats run in bf16; final residual/LN in fp32.

fp8 weights are pre-scaled on host (x32 / x1024) to dodge e4m3 subnormals;
descales are folded into per-partition scalars downstream.

Linear-algebra folds (host, exact):
  - memory layernorm is applied via rank-1 corrections that ride the K/V
    projections as an extra DoubleRow contraction pair (row 0 = 32*mu vs
    negated weight rowsums); the K-side bias cancels in softmax; the V-side
    bias is folded into the gate/integration biases (sum(attn)=1).
  - query_proj+Wq fused; out_w folded into gate/integration weights.
"""

import sys
from contextlib import ExitStack

if "/opt/trn_rl_repo" not in sys.path:
    sys.path.insert(0, "/opt/trn_rl_repo")

import numpy as np
import ml_dtypes

import concourse.bass as bass
import concourse.mybir as mybir
import concourse.tile as tile
from concourse import bacc
from concourse.bass_utils import run_bass_kernel_spmd
from concourse.masks import make_identity

F32 = mybir.dt.float32
F32R = mybir.dt.float32r
BF16 = mybir.dt.bfloat16
FP8 = mybir.dt.float8e4
AF = mybir.ActivationFunctionType
OP = mybir.AluOpType
DR = mybir.MatmulPerfMode.DoubleRow

H = 1024
NH = 4
HD = H // NH          # 256
K = 2048              # top_k
B, S = 4, 2048
N_CORES = 8
R = (B * S) // N_CORES  # 1024 rows per core
EPS = 1e-5
H2 = 2 * H

HC = H // 128         # 8
H2C = H2 // 128       # 16
KC = K // 128         # 16 key chunks
KCP = KC // 2         # 8 key-chunk pairs
RT = R // 512         # 2 row tiles of 512
RT4 = R // 256        # 4 row tiles of 256

SQ = 32.0             # wc fp8 pre-scale
SKV = 32.0            # wk/wv fp8 pre-scale
SGX = 1024.0          # gwx fp8 pre-scale
SGA = 32.0            # gwa' fp8 pre-scale (x32 from ctx8 -> 1024 total)
S1A = 32.0            # w1a' fp8 pre-scale

NP_FP8 = ml_dtypes.float8_e4m3
NP_BF16 = ml_dtypes.bfloat16


def build_program(ln2_trivial):
    nc = bacc.Bacc("TRN2", target_bir_lowering=False)

    def dp(name, shape, dt):
        return nc.declare_dram_parameter(name, shape, dt, isOutput=False)

    d_x8 = dp("x8", [128, HC, R], FP8)
    d_xt = dp("xt", [128, HC, R], F32R)
    d_m8 = dp("m8", [128, HC + 2, K], FP8)
    d_wk8 = dp("wk8", [128, HC, 5, 2, 128], FP8)
    d_wc8 = dp("wc8", [128, HC, 4, 2, 128], FP8)
    d_wv8 = dp("wv8", [128, 5, 2, H], FP8)
    d_gwx8 = dp("gwx8", [128, HC, 4, 2, 128], FP8)
    d_gwa8 = dp("gwa8", [128, HC, 4, 2, 128], FP8)
    d_w1a8 = dp("w1a8", [128, H2C, 4, 2, 128], FP8)
    d_w1x = dp("w1x", [H2C, 128, HC, 128], F32R)
    d_w2b = dp("w2b", [128, HC, H2C, 128], BF16)
    d_cv = dp("cv", [K], F32)       # rstd/SKV
    d_bc = dp("bc", [H], F32)       # fused q bias
    d_gb = dp("gb", [H], F32)
    d_b1 = dp("b1", [H2], F32)
    d_b2 = dp("b2", [H], F32)
    d_ilg = dp("ilg", [H2], F32)
    d_ilb = dp("ilb", [H2], F32)
    d_l2g = dp("l2g", [H], F32)
    d_l2b = dp("l2b", [H], F32)
    d_out = nc.declare_dram_parameter("out", [R, H], F32, isOutput=True)

    def bcast(pool, vec, n, nm):
        t = pool.tile([128, n], F32, tag=f"bc_{nm}", name=f"bc_{nm}")
        nc.sync.dma_start(
            out=t, in_=vec[:].unsqueeze(0).partition_broadcast(128).squeeze(1))
        return t

    def load_pp(pool, vec, n, nm):
        t = pool.tile([128, n], F32, tag=f"pp_{nm}", name=f"pp_{nm}")
        nc.sync.dma_start(out=t, in_=vec[:].rearrange("(c p) -> p c", p=128))
        return t

    with tile.TileContext(nc) as tc, ExitStack() as top:
        singles = top.enter_context(tc.tile_pool(name="singles", bufs=1))
        spw1 = ExitStack()
        pw1 = spw1.enter_context(tc.tile_pool(name="pw1", bufs=4))

        # K8/V8/wc8 live from phase A through the end of attention only.
        # (pool opened before pa: pools must be released in stack order)
        sKV = ExitStack()
        pkv = sKV.enter_context(tc.tile_pool(name="pkv", bufs=1))
        K8 = pkv.tile([128, NH, 2, K], FP8)
        V8 = pkv.tile([128, KCP, 2, H], FP8)
        wc8 = pkv.tile([128, HC, 4, 2, 128], FP8)

        # ---- phase A inputs first: the PE's first work depends on them
        sA = ExitStack()
        pa = sA.enter_context(tc.tile_pool(name="pa", bufs=1))
        pa_ps = sA.enter_context(
            tc.tile_pool(name="pa_ps", bufs=1, space="PSUM"))
        mem8 = pa.tile([128, HC + 2, K], FP8)
        wk8 = pa.tile([128, HC, 5, 2, 128], FP8)
        wv8 = pa.tile([128, 5, 2, H], FP8)
        for g in range(4):
            nc.sync.dma_start(out=wk8[:, 2 * g:2 * g + 2],
                              in_=d_wk8[:, 2 * g:2 * g + 2])
        for kt in range(4):
            ksl = bass.ts(kt, 512)
            nc.sync.dma_start(out=mem8[:, :, ksl], in_=d_m8[:, :, ksl])
        nc.sync.dma_start(out=wv8, in_=d_wv8[:, :, :, :])
        c_bc = bcast(pa, d_cv, K, "cv")
        cv_pp = load_pp(pa, d_cv, KC, "cvp")

        nc.sync.dma_start(out=wc8, in_=d_wc8[:, :, :, :, :])

        x8 = singles.tile([128, HC, R], FP8)
        nc.sync.dma_start(out=x8, in_=d_x8[:, :, :])
        xt = singles.tile([128, HC, R], F32R)
        nc.sync.dma_start(out=xt, in_=d_xt[:, :, :])

        ident = singles.tile([128, 128], F32)
        make_identity(nc, ident)
        scr1 = singles.tile([128, 128], F32)
        nc.vector.memset(scr1, 1.0)
        ones8 = singles.tile([128, 2, 128], FP8)
        nc.scalar.activation(out=ones8[:, 0, :], in_=scr1, func=AF.Copy)
        nc.scalar.activation(out=ones8[:, 1, :], in_=scr1, func=AF.Copy)
        ones2k = singles.tile([128, 128], BF16)
        nc.scalar.activation(out=ones2k, in_=scr1, func=AF.Copy,
                             scale=1.0 / 2048.0)
        eps_t = singles.tile([128, 1], F32)
        nc.vector.memset(eps_t, EPS)

        bc_pp = load_pp(singles, d_bc, HC, "bc")
        gb_pp = load_pp(singles, d_gb, HC, "gb")
        b1_pp = load_pp(singles, d_b1, H2C, "b1")
        b2_pp = load_pp(singles, d_b2, HC, "b2")
        ilg_pp = load_pp(singles, d_ilg, H2C, "ilg")
        ilb_pp = load_pp(singles, d_ilb, H2C, "ilb")

        # long-lived activations
        ctx8 = singles.tile([128, HC, R], FP8)
        h1b = singles.tile([128, H2C, R], BF16)
        mu2_sb = singles.tile([128, R], F32)
        rstd2_sb = singles.tile([128, R], F32)

        w1slab = {}

        def w1_slab(oc2):
            if oc2 not in w1slab:
                sl = pw1.tile([128, HC, 128], F32R, tag="w1x",
                              name=f"w1x{oc2}")
                nc.sync.dma_start(out=sl, in_=d_w1x[oc2])
                w1slab[oc2] = sl
            return w1slab[oc2]

        # =============== Phase A: K/V projections over the bank ============
        # LN1 rank-1 corrections ride as a 5th contraction pair:
        # mem8[:, 8:10] row (0,0) holds 32*mu; wk8[...,4] and wv8[:, 4]
        # hold the negated weight rowsums.
        for kt in range(4):
            ksl = bass.ts(kt, 512)
            for oc in range(HC):
                ps = pa_ps.tile([128, 512], F32, tag="psK", name="psK",
                                bufs=3)
                for i in range(5):
                    nc.tensor.matmul(ps, wk8[:, oc, i],
                                     mem8[:, 2 * i:2 * i + 2, ksl],
                                     start=(i == 0), stop=(i == 4),
                                     perf_mode=DR)
                nc.vector.tensor_mul(K8[:, oc // 2, oc % 2, ksl], ps,
                                     c_bc[:, ksl])
            for kc in range(kt * 4, kt * 4 + 4):
                for fh in range(2):
                    fsl = bass.ts(fh, 512)
                    ps = pa_ps.tile([128, 512], F32, tag="psV",
                                    name="psV", bufs=3)
                    for i in range(5):
                        nc.tensor.matmul(
                            ps, mem8[:, 2 * i:2 * i + 2,
                                     kc * 128:(kc + 1) * 128],
                            wv8[:, i, :, fsl],
                            start=(i == 0), stop=(i == 4), perf_mode=DR)
                    nc.scalar.activation(out=V8[:, kc // 2, kc % 2, fsl],
                                         in_=ps, func=AF.Copy,
                                         scale=cv_pp[:, kc:kc + 1])
        sA.close()

        # =============== Phases B+C: query proj + attention =================
        emitted_h1x = set()
        with ExitStack() as sBC:
            pb = sBC.enter_context(tc.tile_pool(name="pb", bufs=1))
            pb_t = sBC.enter_context(tc.tile_pool(name="pb_t", bufs=4))
            pb_e = sBC.enter_context(tc.tile_pool(name="pb_e", bufs=6))
            pb_ps = sBC.enter_context(
                tc.tile_pool(name="pb_ps", bufs=1, space="PSUM"))

            w1a8 = pb.tile([128, H2C, 4, 2, 128], FP8)
            nc.sync.dma_start(out=w1a8, in_=d_w1a8[:, :, :, :, :])
            Q8 = pb.tile([128, HC, R], FP8)

            def q_group(h, oc, rt):
                rsl = bass.ts(rt, 512)
                ps = pb_ps.tile([128, 512], F32, tag="fill", name="fill",
                                bufs=1)
                for i in range(4):
                    nc.tensor.matmul(ps, wc8[:, oc, i],
                                     x8[:, 2 * i:2 * i + 2, rsl],
                                     start=(i == 0), stop=(i == 3),
                                     perf_mode=DR)
                nc.vector.tensor_scalar(
                    out=Q8[:, oc, rsl], in0=ps, scalar1=1.0 / SQ,
                    scalar2=bc_pp[:, oc:oc + 1], op0=OP.mult, op1=OP.add)

            def h1x_group(oc2, rt, pspool, use_act):
                rsl = bass.ts(rt, 512)
                sl = w1_slab(oc2)
                ps = pspool.tile([128, 512], F32, tag="fill", name="fill",
                                 bufs=1 if pspool is pb_ps else 2)
                for hc in range(HC):
                    nc.tensor.matmul(ps, sl[:, hc], xt[:, hc, rsl],
                                     start=(hc == 0), stop=(hc == HC - 1))
                if use_act:
                    nc.scalar.activation(out=h1b[:, oc2, rsl], in_=ps,
                                         func=AF.Identity,
                                         bias=b1_pp[:, oc2:oc2 + 1])
                else:
                    nc.vector.tensor_scalar(
                        out=h1b[:, oc2, rsl], in0=ps,
                        scalar1=b1_pp[:, oc2:oc2 + 1], scalar2=None,
                        op0=OP.add)
                emitted_h1x.add((oc2, rt))

            fillers = [(q_group, (h, 2 * h + j, rt))
                       for h in range(1, NH) for j in range(2)
                       for rt in range(RT)]
            fillers += [(h1x_group, (oc2, rt, pb_ps, False))
                        for oc2 in range(H2C) for rt in range(RT)]
            fillers.reverse()

            def pop_filler():
                if fillers:
                    fn, args = fillers.pop()
                    fn(*args)

            for j in range(2):
                for rt in range(RT):
                    q_group(0, j, rt)

            for h in range(NH):
                for rt in range(RT):
                    rsl = bass.ts(rt, 512)
                    sums = pb_ps.tile([128, 512], F32, tag="sums",
                                      name="sums", bufs=1)
                    cps = [pb_ps.tile([128, 512], F32, tag=f"ctx{fg}",
                                      name=f"ctx{fg}", bufs=1)
                           for fg in range(2)]
                    for p in range(KCP):
                        e8p = pb_e.tile([128, 2, 512], FP8, tag="e8",
                                        name="e8")
                        for j in range(2):
                            kc = 2 * p + j
                            sc = pb_ps.tile([128, 512], F32, tag="sc",
                                            name="sc", bufs=4)
                            nc.tensor.matmul(
                                sc, K8[:, h, :, kc * 128:(kc + 1) * 128],
                                Q8[:, 2 * h:2 * h + 2, rsl],
                                start=True, stop=True, perf_mode=DR)
                            nc.scalar.activation(out=e8p[:, j, :], in_=sc,
                                                 func=AF.Exp, scale=1.0 / 16)
                        if p % 2 == 1:
                            pop_filler()
                        nc.tensor.matmul(sums, ones8, e8p,
                                         start=(p == 0), stop=(p == KCP - 1),
                                         perf_mode=DR)
                        for fg in range(2):
                            nc.tensor.matmul(
                                cps[fg],
                                V8[:, p, :,
                                   h * HD + fg * 128:h * HD + (fg + 1) * 128],
                                e8p, start=(p == 0), stop=(p == KCP - 1),
                                perf_mode=DR)
                    rec = pb_t.tile([128, 512], F32, tag="rec", name="rec")
                    nc.vector.reciprocal_approx_fast(out=rec, in_=sums)
                    for fg in range(2):
                        nc.vector.scalar_tensor_tensor(
                            out=ctx8[:, 2 * h + fg, rsl], in0=cps[fg],
                            scalar=32.0, in1=rec, op0=OP.mult, op1=OP.mult)

        sKV.close()

        # ====== Phase D1+D2a: finish h1, iLN stats (interleaved) ===========
        with ExitStack() as sD1:
            pd1_t = sD1.enter_context(tc.tile_pool(name="pd1_t", bufs=3))
            pd1_ps = sD1.enter_context(
                tc.tile_pool(name="pd1_ps", bufs=1, space="PSUM"))
            mu2_ps = [pd1_ps.tile([128, 512], F32, tag=f"m2_{i}",
                                  name=f"m2_{i}") for i in range(RT)]
            ms2_ps = [pd1_ps.tile([128, 512], F32, tag=f"s2_{i}",
                                  name=f"s2_{i}") for i in range(RT)]
            for oc2 in range(H2C):
                for rt in range(RT):
                    if (oc2, rt) not in emitted_h1x:
                        h1x_group(oc2, rt, pd1_ps, True)
                    rsl = bass.ts(rt, 512)
                    ps2 = pd1_ps.tile([128, 512], F32, tag="psH", name="psH",
                                      bufs=2)
                    for i in range(4):
                        nc.tensor.matmul(ps2, w1a8[:, oc2, i],
                                         ctx8[:, 2 * i:2 * i + 2, rsl],
                                         start=(i == 0), stop=(i == 3),
                                         perf_mode=DR)
                    nc.vector.scalar_tensor_tensor(
                        out=h1b[:, oc2, rsl], in0=ps2, scalar=1.0 / 1024.0,
                        in1=h1b[:, oc2, rsl], op0=OP.mult, op1=OP.add)
                sq = pd1_t.tile([128, R], BF16, tag="sq", name="sq")
                eng = nc.vector if oc2 % 2 == 0 else nc.gpsimd
                eng.tensor_mul(sq, h1b[:, oc2, :], h1b[:, oc2, :])
                for rt in range(RT):
                    rsl = bass.ts(rt, 512)
                    nc.tensor.matmul(mu2_ps[rt], ones2k, h1b[:, oc2, rsl],
                                     start=(oc2 == 0), stop=(oc2 == H2C - 1))
                    nc.tensor.matmul(ms2_ps[rt], ones2k, sq[:, rsl],
                                     start=(oc2 == 0), stop=(oc2 == H2C - 1))
            for rt in range(RT):
                rsl = bass.ts(rt, 512)
                nc.scalar.activation(out=mu2_sb[:, rsl], in_=mu2_ps[rt],
                                     func=AF.Copy)
                var = pd1_t.tile([128, 512], F32, tag="var", name="var")
                nc.vector.tensor_mul(var, mu2_sb[:, rsl], mu2_sb[:, rsl])
                nc.vector.tensor_sub(var, ms2_ps[rt], var)
                nc.scalar.activation(out=var, in_=var, func=AF.Ln,
                                     bias=eps_t, scale=1.0)
                nc.scalar.activation(out=rstd2_sb[:, rsl], in_=var,
                                     func=AF.Exp, scale=-0.5)

        spw1.close()

        def apply_gelu(pool, rt):
            rsl = bass.ts(rt, 512)
            for oc2 in range(H2C):
                td = pool.tile([128, 512], F32, tag="td", name="td")
                eng = nc.vector if oc2 % 2 == 0 else nc.gpsimd
                eng.tensor_sub(td, h1b[:, oc2, rsl], mu2_sb[:, rsl])
                nc.vector.scalar_tensor_tensor(
                    out=td, in0=td, scalar=ilg_pp[:, oc2:oc2 + 1],
                    in1=rstd2_sb[:, rsl], op0=OP.mult, op1=OP.mult)
                nc.scalar.activation(out=h1b[:, oc2, rsl], in_=td,
                                     func=AF.Gelu,
                                     bias=ilb_pp[:, oc2:oc2 + 1])

        # =============== Phase D3+D4: gate, integ, residual, LN2 ===========
        with ExitStack() as sD3:
            pd3 = sD3.enter_context(tc.tile_pool(name="pd3", bufs=1))
            pd3_t = sD3.enter_context(tc.tile_pool(name="pd3_t", bufs=3))
            pd3_y = sD3.enter_context(tc.tile_pool(name="pd3_y", bufs=2))
            pd3_o = sD3.enter_context(tc.tile_pool(name="pd3_o", bufs=2))
            pd3_ps = sD3.enter_context(
                tc.tile_pool(name="pd3_ps", bufs=1, space="PSUM"))
            pd3_pt = sD3.enter_context(
                tc.tile_pool(name="pd3_pt", bufs=2, space="PSUM"))
            w2b = pd3.tile([128, HC, H2C, 128], BF16)
            nc.sync.dma_start(out=w2b, in_=d_w2b[:, :, :, :])
            gwx8 = pd3.tile([128, HC, 4, 2, 128], FP8)
            nc.sync.dma_start(out=gwx8, in_=d_gwx8[:, :, :, :, :])
            gwa8 = pd3.tile([128, HC, 4, 2, 128], FP8)
            nc.sync.dma_start(out=gwa8, in_=d_gwa8[:, :, :, :, :])
            if not ln2_trivial:
                l2g_bc = bcast(pd3, d_l2g, H, "l2g")
                l2b_bc = bcast(pd3, d_l2b, H, "l2b")

            apply_gelu(pd3_t, 0)
            apply_gelu(pd3_t, 1)

            def d4_chunk(yt, rt4, rc2):
                row0 = rt4 * 256 + rc2 * 128
                tp = pd3_pt.tile([128, H], F32, tag="tp", name="tp")
                for oc in range(HC):
                    nc.tensor.transpose(
                        tp[:, oc * 128:(oc + 1) * 128],
                        yt[:, oc, rc2 * 128:(rc2 + 1) * 128], ident)
                bst = pd3_o.tile([128, 2, 6], F32, tag="bst", name="bst")
                for i in range(2):
                    nc.vector.bn_stats(out=bst[:, i, :],
                                       in_=tp[:, i * 512:(i + 1) * 512])
                mv = pd3_o.tile([128, 2], F32, tag="mv", name="mv")
                nc.vector.bn_aggr(out=mv, in_=bst)
                sd = pd3_o.tile([128, 1], F32, tag="sd", name="sd")
                nc.scalar.activation(out=sd, in_=mv[:, 1:2], func=AF.Sqrt,
                                     bias=eps_t, scale=1.0)
                rstd = pd3_o.tile([128, 1], F32, tag="rsd", name="rsd")
                nc.vector.reciprocal(out=rstd, in_=sd)
                nmr = pd3_o.tile([128, 1], F32, tag="nmr", name="nmr")
                nc.vector.scalar_tensor_tensor(
                    out=nmr, in0=mv[:, 0:1], scalar=-1.0, in1=rstd,
                    op0=OP.mult, op1=OP.mult)
                yr = pd3_o.tile([128, H], F32, tag="yr", name="yr")
                for i in range(2):
                    nc.scalar.activation(out=yr[:, i * 512:(i + 1) * 512],
                                         in_=tp[:, i * 512:(i + 1) * 512],
                                         func=AF.Identity, bias=nmr,
                                         scale=rstd)
                if not ln2_trivial:
                    nc.vector.tensor_mul(yr, yr, l2g_bc)
                    nc.vector.tensor_add(yr, yr, l2b_bc)
                nc.sync.dma_start(out=d_out[row0:row0 + 128, :], in_=yr)

            for rt4 in range(RT4):
                r0 = rt4 * 256
                rsl = slice(r0, r0 + 256)
                yt = pd3_y.tile([128, HC, 256], F32, tag="yt", name="yt")
                for oc in range(HC):
                    gps = pd3_ps.tile([128, 256], F32, tag="gps", name="gps",
                                      bufs=2)
                    for i in range(4):
                        nc.tensor.matmul(gps, gwx8[:, oc, i],
                                         x8[:, 2 * i:2 * i + 2, rsl],
                                         start=(i == 0), stop=False,
                                         perf_mode=DR)
                    for i in range(4):
                        nc.tensor.matmul(gps, gwa8[:, oc, i],
                                         ctx8[:, 2 * i:2 * i + 2, rsl],
                                         start=False, stop=(i == 3),
                                         perf_mode=DR)
                    sig = pd3_t.tile([128, 256], F32, tag="sig", name="sig")
                    nc.scalar.activation(out=sig, in_=gps, func=AF.Sigmoid,
                                         bias=gb_pp[:, oc:oc + 1],
                                         scale=1.0 / SGX)
                    ips = pd3_ps.tile([128, 256], F32, tag="ips", name="ips",
                                      bufs=2)
                    for hc2 in range(H2C):
                        nc.tensor.matmul(ips, w2b[:, oc, hc2],
                                         h1b[:, hc2, rsl],
                                         start=(hc2 == 0),
                                         stop=(hc2 == H2C - 1))
                    tmp = pd3_t.tile([128, 256], F32, tag="ytmp", name="ytmp")
                    nc.vector.scalar_tensor_tensor(
                        out=tmp, in0=ips, scalar=b2_pp[:, oc:oc + 1],
                        in1=sig, op0=OP.add, op1=OP.mult)
                    nc.vector.tensor_add(yt[:, oc, :], tmp,
                                         xt[:, oc, rsl].bitcast(F32))
                for rc2 in range(2):
                    d4_chunk(yt, rt4, rc2)

    nc.compile()
    return nc


_NC_CACHE = {}


def _get_nc(ln2_trivial):
    if ln2_trivial not in _NC_CACHE:
        _NC_CACHE[ln2_trivial] = build_program(ln2_trivial)
    return _NC_CACHE[ln2_trivial]


def kernel(query_hidden, mem_keys, importance, recency, access_count,
           Wq, bq, in_w, in_b, out_w, out_b, gate_w, gate_b,
           int_w1, int_b1, int_ln_g, int_ln_b, int_w2, int_b2,
           ln1_g, ln1_b, ln2_g, ln2_b, sel_params, top_k):
    np32 = lambda a: np.asarray(a, dtype=np.float32)
    f8 = lambda a: np.ascontiguousarray(a.astype(NP_FP8))
    bf = lambda a: np.ascontiguousarray(a.astype(NP_BF16))

    query_hidden = np32(query_hidden)
    mem_keys = np32(mem_keys)
    top_k = int(top_k)
    assert top_k == K, f"kernel compiled for top_k={K}, got {top_k}"

    # HTPS selection (host): softmax-weighted score, top-k set, gather.
    sp = np32(sel_params)
    w = np.exp(sp - sp.max())
    w = w / w.sum()
    acc = np32(access_count)
    sel = (w[0] * np32(importance) + w[1] * np32(recency)
           + w[2] * (acc / acc.max()))
    idx = np.argpartition(-sel, top_k - 1)[:top_k]
    mem = mem_keys[idx]                               # [K, H]

    mu = mem.mean(1)
    rstd = 1.0 / np.sqrt(mem.var(1) + EPS)

    in_w = np32(in_w)
    in_b = np32(in_b)
    wq, wk, wv = in_w[:H], in_w[H:2 * H], in_w[2 * H:]
    bqi, bki, bvi = in_b[:H], in_b[H:2 * H], in_b[2 * H:]
    g1 = np32(ln1_g)
    b1v = np32(ln1_b)

    wkE = wk * g1[None, :]
    wvE = wv * g1[None, :]
    wc = wq @ np32(Wq)
    bcv = wq @ np32(bq) + bqi

    out_w = np32(out_w)
    gate_w = np32(gate_w)
    int_w1 = np32(int_w1)
    gwx, gwa = gate_w[:, :H], gate_w[:, H:]
    w1x, w1a = int_w1[:, :H], int_w1[:, H:]
    bvF = bvi + wv @ b1v                              # V bias (exact)
    out_bF = out_w @ bvF + np32(out_b)
    gate_bF = np32(gate_b) + gwa @ out_bF
    int_b1F = np32(int_b1) + w1a @ out_bF
    gwaE = gwa @ out_w
    w1aE = w1a @ out_w

    def chunk5(wmat, scale):
        # [O, F] -> fp8 [128, O//128, 4, 2, 128] (pair-sliced stationary)
        o, f = wmat.shape
        a = np.ascontiguousarray(wmat.T) * scale      # [F, O]
        a = a.reshape(4, 2, 128, o // 128, 128)       # f = (2i+j)*128 + p
        return f8(np.ascontiguousarray(a.transpose(2, 3, 0, 1, 4)))

    X = query_hidden.reshape(B * S, H)

    def fm(a, dtype_cast):
        # [N, 128*c] -> feature-major [128, c, N]
        n, ftot = a.shape
        t = np.ascontiguousarray(a.T).reshape(ftot // 128, 128, n)
        return dtype_cast(np.ascontiguousarray(t.transpose(1, 0, 2)))

    # extended mem: chunks 8-9 are the LN-correction pair, row (0,0)=32*mu
    m8e = np.zeros((128, HC + 2, K), dtype=NP_FP8)
    m8e[:, :HC, :] = fm(mem, f8)
    m8e[0, HC, :] = (SKV * mu).astype(NP_FP8)

    wk8e = np.zeros((128, HC, 5, 2, 128), dtype=NP_FP8)
    wk8e[:, :, :4] = chunk5(wkE, SKV)
    wk8e[0, :, 4, 0, :] = (-wkE.sum(1)).astype(NP_FP8).reshape(HC, 128)

    wv8e = np.zeros((128, 5, 2, H), dtype=NP_FP8)
    wv8e[:, :4] = f8(np.ascontiguousarray(
        (wvE.T * SKV).reshape(4, 2, 128, H).transpose(2, 0, 1, 3)))
    wv8e[0, 4, 0, :] = (-wvE.sum(1)).astype(NP_FP8)

    common = {
        "m8": m8e,
        "wk8": wk8e,
        "wc8": chunk5(wc, SQ),
        "wv8": wv8e,
        "gwx8": chunk5(gwx, SGX),
        "gwa8": chunk5(gwaE, SGA),
        "w1a8": chunk5(w1aE, S1A),
        "w1x": np.ascontiguousarray(
            w1x.T.reshape(HC, 128, H2C, 128).transpose(2, 1, 0, 3)),
        "w2b": bf(np.ascontiguousarray(
            np32(int_w2).T.reshape(H2C, 128, HC, 128).transpose(1, 2, 0, 3))),
        "cv": rstd / SKV,
        "bc": bcv,
        "gb": gate_bF,
        "b1": int_b1F,
        "b2": np32(int_b2),
        "ilg": np32(int_ln_g),
        "ilb": np32(int_ln_b),
        "l2g": np32(ln2_g),
        "l2b": np32(ln2_b),
    }

    ln2_trivial = bool(np.all(np32(ln2_g) == 1.0)
                       and np.all(np32(ln2_b) == 0.0))

    in_maps = []
    for c in range(N_CORES):
        m = dict(common)
        Xc = X[c * R:(c + 1) * R]
        m["x8"] = fm(Xc, f8)
        m["xt"] = fm(Xc, lambda a: np.ascontiguousarray(a))
        in_maps.append(m)

    nc = _get_nc(ln2_trivial)
    res = run_bass_kernel_spmd(nc, in_maps, core_ids=list(range(N_CORES)))
    out = np.empty((B * S, H), dtype=np.float32)
    for c in range(N_CORES):
        out[c * R:(c + 1) * R] = res.results[c]["out"]
    return out.reshape(B, S, H)


# revision 19
# speedup vs baseline: 1.9471x; 1.0315x over previous
"""MemoryRetriever kernel for 8x Trainium2 NeuronCores — fp8 DoubleRow edition.

Data-parallel over the B*S=8192 query rows (1024 rows/core); the selected
memory bank and all weights are replicated.

Precision plan (validated against the reference on host, maxrel ~1e-2):
  - attention block (K/V/Q projections, scores, softmax, ctx) and the gate
    run in fp8e4m3 DoubleRow matmuls (2.07x fp32r throughput measured);
  - the x-side of the integration MLP (h1x) runs in fp32r, the ctx side in
    fp8 (ctx is tiny so its quantization error is negligible);
  - integ + LN stats run in bf16; final residual/LN in fp32.

fp8 weights are pre-scaled on host (x32 / x1024) to dodge e4m3 subnormals;
descales are folded into per-partition scalars downstream.

Linear-algebra folds (host, exact):
  - memory layernorm is applied via rank-1 corrections that ride the K/V
    projections as an extra DoubleRow contraction pair (row 0 = 32*mu vs
    negated weight rowsums); the K-side bias cancels in softmax; the V-side
    bias is folded into the gate/integration biases (sum(attn)=1).
  - query_proj+Wq fused; out_w folded into gate/integration weights.
"""

import sys
from contextlib import ExitStack

if "/opt/trn_rl_repo" not in sys.path:
    sys.path.insert(0, "/opt/trn_rl_repo")

import numpy as np
import ml_dtypes

import concourse.bass as bass
import concourse.mybir as mybir
import concourse.tile as tile
from concourse import bacc
from concourse.bass_utils import run_bass_kernel_spmd
from concourse.masks import make_identity

F32 = mybir.dt.float32
F32R = mybir.dt.float32r
BF16 = mybir.dt.bfloat16
FP8 = mybir.dt.float8e4
AF = mybir.ActivationFunctionType
OP = mybir.AluOpType
DR = mybir.MatmulPerfMode.DoubleRow

H = 1024
NH = 4
HD = H // NH          # 256
K = 2048              # top_k
B, S = 4, 2048
N_CORES = 8
R = (B * S) // N_CORES  # 1024 rows per core
EPS = 1e-5
H2 = 2 * H

HC = H // 128         # 8
H2C = H2 // 128       # 16
KC = K // 128         # 16 key chunks
KCP = KC // 2         # 8 key-chunk pairs
RT = R // 512         # 2 row tiles of 512
RT4 = R // 256        # 4 row tiles of 256

SQ = 32.0             # wc fp8 pre-scale
SKV = 32.0            # wk/wv fp8 pre-scale
SGX = 1024.0          # gwx fp8 pre-scale
SGA = 32.0            # gwa' fp8 pre-scale (x32 from ctx8 -> 1024 total)
S1A = 32.0            # w1a' fp8 pre-scale

NP_FP8 = ml_dtypes.float8_e4m3
NP_BF16 = ml_dtypes.bfloat16


def build_program(ln2_trivial):
    nc = bacc.Bacc("TRN2", target_bir_lowering=False)

    def dp(name, shape, dt):
        return nc.declare_dram_parameter(name, shape, dt, isOutput=False)

    d_x8 = dp("x8", [128, HC, R], FP8)
    d_xt = dp("xt", [128, HC, R], F32R)
    d_m8 = dp("m8", [128, HC + 2, K], FP8)
    d_wk8 = dp("wk8", [128, HC, 5, 2, 128], FP8)
    d_wc8 = dp("wc8", [128, HC, 4, 2, 128], FP8)
    d_wv8 = dp("wv8", [128, 5, 2, H], FP8)
    d_gwx8 = dp("gwx8", [128, HC, 4, 2, 128], FP8)
    d_gwa8 = dp("gwa8", [128, HC, 4, 2, 128], FP8)
    d_w1a8 = dp("w1a8", [128, H2C, 4, 2, 128], FP8)
    d_w1x = dp("w1x", [H2C, 128, HC, 128], F32R)
    d_w2b = dp("w2b", [128, HC, H2C, 128], BF16)
    d_cv = dp("cv", [K], F32)       # rstd/SKV
    d_bc = dp("bc", [H], F32)       # fused q bias
    d_gb = dp("gb", [H], F32)
    d_b1 = dp("b1", [H2], F32)
    d_b2 = dp("b2", [H], F32)
    d_ilg = dp("ilg", [H2], F32)
    d_ilb = dp("ilb", [H2], F32)
    d_l2g = dp("l2g", [H], F32)
    d_l2b = dp("l2b", [H], F32)
    d_out = nc.declare_dram_parameter("out", [R, H], F32, isOutput=True)

    def bcast(pool, vec, n, nm):
        t = pool.tile([128, n], F32, tag=f"bc_{nm}", name=f"bc_{nm}")
        nc.sync.dma_start(
            out=t, in_=vec[:].unsqueeze(0).partition_broadcast(128).squeeze(1))
        return t

    def load_pp(pool, vec, n, nm):
        t = pool.tile([128, n], F32, tag=f"pp_{nm}", name=f"pp_{nm}")
        nc.sync.dma_start(out=t, in_=vec[:].rearrange("(c p) -> p c", p=128))
        return t

    with tile.TileContext(nc) as tc, ExitStack() as top:
        singles = top.enter_context(tc.tile_pool(name="singles", bufs=1))
        spw1 = ExitStack()
        pw1 = spw1.enter_context(tc.tile_pool(name="pw1", bufs=4))

        # K8/V8/wc8 live from phase A through the end of attention only.
        # (pool opened before pa: pools must be released in stack order)
        sKV = ExitStack()
        pkv = sKV.enter_context(tc.tile_pool(name="pkv", bufs=1))
        K8 = pkv.tile([128, NH, 2, K], FP8)
        V8 = pkv.tile([128, KCP, 2, H], FP8)
        wc8 = pkv.tile([128, HC, 4, 2, 128], FP8)

        # ---- phase A inputs first: the PE's first work depends on them
        sA = ExitStack()
        pa = sA.enter_context(tc.tile_pool(name="pa", bufs=1))
        pa_ps = sA.enter_context(
            tc.tile_pool(name="pa_ps", bufs=1, space="PSUM"))
        mem8 = pa.tile([128, HC + 2, K], FP8)
        wk8 = pa.tile([128, HC, 5, 2, 128], FP8)
        wv8 = pa.tile([128, 5, 2, H], FP8)
        for g in range(4):
            nc.sync.dma_start(out=wk8[:, 2 * g:2 * g + 2],
                              in_=d_wk8[:, 2 * g:2 * g + 2])
        for kt in range(4):
            ksl = bass.ts(kt, 512)
            nc.sync.dma_start(out=mem8[:, :, ksl], in_=d_m8[:, :, ksl])
        for g in range(2):
            nc.sync.dma_start(out=wv8[:, :, :, g * 512:(g + 1) * 512],
                              in_=d_wv8[:, :, :, g * 512:(g + 1) * 512])
        c_bc = bcast(pa, d_cv, K, "cv")
        cv_pp = load_pp(pa, d_cv, KC, "cvp")

        nc.sync.dma_start(out=wc8, in_=d_wc8[:, :, :, :, :])

        x8 = singles.tile([128, HC, R], FP8)
        nc.sync.dma_start(out=x8, in_=d_x8[:, :, :])
        xt = singles.tile([128, HC, R], F32R)
        nc.sync.dma_start(out=xt, in_=d_xt[:, :, :])

        ident = singles.tile([128, 128], F32)
        make_identity(nc, ident)
        scr1 = singles.tile([128, 128], F32)
        nc.vector.memset(scr1, 1.0)
        ones8 = singles.tile([128, 2, 128], FP8)
        nc.scalar.activation(out=ones8[:, 0, :], in_=scr1, func=AF.Copy)
        nc.scalar.activation(out=ones8[:, 1, :], in_=scr1, func=AF.Copy)
        ones2k = singles.tile([128, 128], BF16)
        nc.scalar.activation(out=ones2k, in_=scr1, func=AF.Copy,
                             scale=1.0 / 2048.0)
        eps_t = singles.tile([128, 1], F32)
        nc.vector.memset(eps_t, EPS)

        bc_pp = load_pp(singles, d_bc, HC, "bc")
        gb_pp = load_pp(singles, d_gb, HC, "gb")
        b1_pp = load_pp(singles, d_b1, H2C, "b1")
        b2_pp = load_pp(singles, d_b2, HC, "b2")
        ilg_pp = load_pp(singles, d_ilg, H2C, "ilg")
        ilb_pp = load_pp(singles, d_ilb, H2C, "ilb")

        # long-lived activations
        ctx8 = singles.tile([128, HC, R], FP8)
        h1b = singles.tile([128, H2C, R], BF16)
        mu2_sb = singles.tile([128, R], F32)
        rstd2_sb = singles.tile([128, R], F32)

        w1slab = {}

        def w1_slab(oc2):
            if oc2 not in w1slab:
                sl = pw1.tile([128, HC, 128], F32R, tag="w1x",
                              name=f"w1x{oc2}")
                nc.sync.dma_start(out=sl, in_=d_w1x[oc2])
                w1slab[oc2] = sl
            return w1slab[oc2]

        # =============== Phase A: K/V projections over the bank ============
        # LN1 rank-1 corrections ride as a 5th contraction pair:
        # mem8[:, 8:10] row (0,0) holds 32*mu; wk8[...,4] and wv8[:, 4]
        # hold the negated weight rowsums.
        def a_kgroup(kt):
            ksl = bass.ts(kt, 512)
            for oc in range(HC):
                ps = pa_ps.tile([128, 512], F32, tag="psK", name="psK",
                                bufs=3)
                for i in range(5):
                    nc.tensor.matmul(ps, wk8[:, oc, i],
                                     mem8[:, 2 * i:2 * i + 2, ksl],
                                     start=(i == 0), stop=(i == 4),
                                     perf_mode=DR)
                nc.vector.tensor_mul(K8[:, oc // 2, oc % 2, ksl], ps,
                                     c_bc[:, ksl])

        def a_vgroup(kt):
            for kc in range(kt * 4, kt * 4 + 4):
                for fh in range(2):
                    fsl = bass.ts(fh, 512)
                    ps = pa_ps.tile([128, 512], F32, tag="psV",
                                    name="psV", bufs=3)
                    for i in range(5):
                        nc.tensor.matmul(
                            ps, mem8[:, 2 * i:2 * i + 2,
                                     kc * 128:(kc + 1) * 128],
                            wv8[:, i, :, fsl],
                            start=(i == 0), stop=(i == 4), perf_mode=DR)
                    nc.scalar.activation(out=V8[:, kc // 2, kc % 2, fsl],
                                         in_=ps, func=AF.Copy,
                                         scale=cv_pp[:, kc:kc + 1])

        # K first while the wv8 DMA lands; then interleave
        a_kgroup(0)
        a_kgroup(1)
        a_vgroup(0)
        a_kgroup(2)
        a_vgroup(1)
        a_kgroup(3)
        a_vgroup(2)
        a_vgroup(3)
        sA.close()

        # =============== Phases B+C: query proj + attention =================
        emitted_h1x = set()
        with ExitStack() as sBC:
            pb = sBC.enter_context(tc.tile_pool(name="pb", bufs=1))
            pb_t = sBC.enter_context(tc.tile_pool(name="pb_t", bufs=4))
            pb_e = sBC.enter_context(tc.tile_pool(name="pb_e", bufs=6))
            pb_ps = sBC.enter_context(
                tc.tile_pool(name="pb_ps", bufs=1, space="PSUM"))

            w1a8 = pb.tile([128, H2C, 4, 2, 128], FP8)
            nc.sync.dma_start(out=w1a8, in_=d_w1a8[:, :, :, :, :])
            Q8 = pb.tile([128, HC, R], FP8)

            def q_group(h, oc, rt):
                rsl = bass.ts(rt, 512)
                ps = pb_ps.tile([128, 512], F32, tag="fill", name="fill",
                                bufs=1)
                for i in range(4):
                    nc.tensor.matmul(ps, wc8[:, oc, i],
                                     x8[:, 2 * i:2 * i + 2, rsl],
                                     start=(i == 0), stop=(i == 3),
                                     perf_mode=DR)
                nc.vector.tensor_scalar(
                    out=Q8[:, oc, rsl], in0=ps, scalar1=1.0 / SQ,
                    scalar2=bc_pp[:, oc:oc + 1], op0=OP.mult, op1=OP.add)

            def h1x_group(oc2, rt, pspool, use_act):
                rsl = bass.ts(rt, 512)
                sl = w1_slab(oc2)
                ps = pspool.tile([128, 512], F32, tag="fill", name="fill",
                                 bufs=1 if pspool is pb_ps else 2)
                for hc in range(HC):
                    nc.tensor.matmul(ps, sl[:, hc], xt[:, hc, rsl],
                                     start=(hc == 0), stop=(hc == HC - 1))
                if use_act:
                    nc.scalar.activation(out=h1b[:, oc2, rsl], in_=ps,
                                         func=AF.Identity,
                                         bias=b1_pp[:, oc2:oc2 + 1])
                else:
                    nc.vector.tensor_scalar(
                        out=h1b[:, oc2, rsl], in0=ps,
                        scalar1=b1_pp[:, oc2:oc2 + 1], scalar2=None,
                        op0=OP.add)
                emitted_h1x.add((oc2, rt))

            fillers = [(q_group, (h, 2 * h + j, rt))
                       for h in range(1, NH) for j in range(2)
                       for rt in range(RT)]
            fillers += [(h1x_group, (oc2, rt, pb_ps, False))
                        for oc2 in range(H2C) for rt in range(RT)]
            fillers.reverse()

            def pop_filler():
                if fillers:
                    fn, args = fillers.pop()
                    fn(*args)

            for j in range(2):
                for rt in range(RT):
                    q_group(0, j, rt)

            for h in range(NH):
                for rt in range(RT):
                    rsl = bass.ts(rt, 512)
                    sums = pb_ps.tile([128, 512], F32, tag="sums",
                                      name="sums", bufs=1)
                    cps = [pb_ps.tile([128, 512], F32, tag=f"ctx{fg}",
                                      name=f"ctx{fg}", bufs=1)
                           for fg in range(2)]
                    e8ps = {}

                    def sc_pair(p):
                        e8p = pb_e.tile([128, 2, 512], FP8, tag="e8",
                                        name="e8")
                        e8ps[p] = e8p
                        for j in range(2):
                            kc = 2 * p + j
                            sc = pb_ps.tile([128, 512], F32, tag="sc",
                                            name="sc", bufs=4)
                            nc.tensor.matmul(
                                sc, K8[:, h, :, kc * 128:(kc + 1) * 128],
                                Q8[:, 2 * h:2 * h + 2, rsl],
                                start=True, stop=True, perf_mode=DR)
                            nc.scalar.activation(out=e8p[:, j, :], in_=sc,
                                                 func=AF.Exp, scale=1.0 / 16)

                    sc_pair(0)
                    for p in range(KCP):
                        if p + 1 < KCP:
                            sc_pair(p + 1)
                        if p % 3 == 2:
                            pop_filler()
                        e8p = e8ps.pop(p)
                        nc.tensor.matmul(sums, ones8, e8p,
                                         start=(p == 0), stop=(p == KCP - 1),
                                         perf_mode=DR)
                        for fg in range(2):
                            nc.tensor.matmul(
                                cps[fg],
                                V8[:, p, :,
                                   h * HD + fg * 128:h * HD + (fg + 1) * 128],
                                e8p, start=(p == 0), stop=(p == KCP - 1),
                                perf_mode=DR)
                    rec = pb_t.tile([128, 512], F32, tag="rec", name="rec")
                    nc.vector.reciprocal_approx_fast(out=rec, in_=sums)
                    for fg in range(2):
                        nc.vector.scalar_tensor_tensor(
                            out=ctx8[:, 2 * h + fg, rsl], in0=cps[fg],
                            scalar=32.0, in1=rec, op0=OP.mult, op1=OP.mult)

        sKV.close()

        # ====== Phase D1+D2a: finish h1, iLN stats (interleaved) ===========
        with ExitStack() as sD1:
            pd1_t = sD1.enter_context(tc.tile_pool(name="pd1_t", bufs=3))
            pd1_ps = sD1.enter_context(
                tc.tile_pool(name="pd1_ps", bufs=1, space="PSUM"))
            mu2_ps = [pd1_ps.tile([128, 512], F32, tag=f"m2_{i}",
                                  name=f"m2_{i}") for i in range(RT)]
            ms2_ps = [pd1_ps.tile([128, 512], F32, tag=f"s2_{i}",
                                  name=f"s2_{i}") for i in range(RT)]
            for oc2 in range(H2C):
                for rt in range(RT):
                    if (oc2, rt) not in emitted_h1x:
                        h1x_group(oc2, rt, pd1_ps, True)
                    rsl = bass.ts(rt, 512)
                    ps2 = pd1_ps.tile([128, 512], F32, tag="psH", name="psH",
                                      bufs=2)
                    for i in range(4):
                        nc.tensor.matmul(ps2, w1a8[:, oc2, i],
                                         ctx8[:, 2 * i:2 * i + 2, rsl],
                                         start=(i == 0), stop=(i == 3),
                                         perf_mode=DR)
                    nc.vector.scalar_tensor_tensor(
                        out=h1b[:, oc2, rsl], in0=ps2, scalar=1.0 / 1024.0,
                        in1=h1b[:, oc2, rsl], op0=OP.mult, op1=OP.add)
                sq = pd1_t.tile([128, R], BF16, tag="sq", name="sq")
                eng = nc.vector if oc2 % 2 == 0 else nc.gpsimd
                eng.tensor_mul(sq, h1b[:, oc2, :], h1b[:, oc2, :])
                for rt in range(RT):
                    rsl = bass.ts(rt, 512)
                    nc.tensor.matmul(mu2_ps[rt], ones2k, h1b[:, oc2, rsl],
                                     start=(oc2 == 0), stop=(oc2 == H2C - 1))
                    nc.tensor.matmul(ms2_ps[rt], ones2k, sq[:, rsl],
                                     start=(oc2 == 0), stop=(oc2 == H2C - 1))
            for rt in range(RT):
                rsl = bass.ts(rt, 512)
                nc.scalar.activation(out=mu2_sb[:, rsl], in_=mu2_ps[rt],
                                     func=AF.Copy)
                var = pd1_t.tile([128, 512], F32, tag="var", name="var")
                nc.vector.tensor_mul(var, mu2_sb[:, rsl], mu2_sb[:, rsl])
                nc.vector.tensor_sub(var, ms2_ps[rt], var)
                nc.scalar.activation(out=var, in_=var, func=AF.Ln,
                                     bias=eps_t, scale=1.0)
                nc.scalar.activation(out=rstd2_sb[:, rsl], in_=var,
                                     func=AF.Exp, scale=-0.5)

        spw1.close()

        def apply_gelu(pool, rt):
            rsl = bass.ts(rt, 512)
            for oc2 in range(H2C):
                td = pool.tile([128, 512], F32, tag="td", name="td")
                eng = nc.vector if oc2 % 2 == 0 else nc.gpsimd
                eng.tensor_sub(td, h1b[:, oc2, rsl], mu2_sb[:, rsl])
                nc.vector.scalar_tensor_tensor(
                    out=td, in0=td, scalar=ilg_pp[:, oc2:oc2 + 1],
                    in1=rstd2_sb[:, rsl], op0=OP.mult, op1=OP.mult)
                nc.scalar.activation(out=h1b[:, oc2, rsl], in_=td,
                                     func=AF.Gelu,
                                     bias=ilb_pp[:, oc2:oc2 + 1])

        # =============== Phase D3+D4: gate, integ, residual, LN2 ===========
        with ExitStack() as sD3:
            pd3 = sD3.enter_context(tc.tile_pool(name="pd3", bufs=1))
            pd3_t = sD3.enter_context(tc.tile_pool(name="pd3_t", bufs=3))
            pd3_y = sD3.enter_context(tc.tile_pool(name="pd3_y", bufs=2))
            pd3_o = sD3.enter_context(tc.tile_pool(name="pd3_o", bufs=2))
            pd3_ps = sD3.enter_context(
                tc.tile_pool(name="pd3_ps", bufs=1, space="PSUM"))
            pd3_pt = sD3.enter_context(
                tc.tile_pool(name="pd3_pt", bufs=2, space="PSUM"))
            pd3_w = sD3.enter_context(tc.tile_pool(name="pd3_w", bufs=3))
            gwx8 = pd3.tile([128, HC, 4, 2, 128], FP8)
            nc.sync.dma_start(out=gwx8, in_=d_gwx8[:, :, :, :, :])
            gwa8 = pd3.tile([128, HC, 4, 2, 128], FP8)
            nc.sync.dma_start(out=gwa8, in_=d_gwa8[:, :, :, :, :])
            if not ln2_trivial:
                l2g_bc = bcast(pd3, d_l2g, H, "l2g")
                l2b_bc = bcast(pd3, d_l2b, H, "l2b")

            apply_gelu(pd3_t, 0)
            apply_gelu(pd3_t, 1)

            def d4_chunk(yt, rt, rc2):
                row0 = rt * 512 + rc2 * 128
                tp = pd3_pt.tile([128, H], F32, tag="tp", name="tp")
                for oc in range(HC):
                    nc.tensor.transpose(
                        tp[:, oc * 128:(oc + 1) * 128],
                        yt[:, oc, rc2 * 128:(rc2 + 1) * 128], ident)
                bst = pd3_o.tile([128, 2, 6], F32, tag="bst", name="bst")
                for i in range(2):
                    nc.vector.bn_stats(out=bst[:, i, :],
                                       in_=tp[:, i * 512:(i + 1) * 512])
                mv = pd3_o.tile([128, 2], F32, tag="mv", name="mv")
                nc.vector.bn_aggr(out=mv, in_=bst)
                sd = pd3_o.tile([128, 1], F32, tag="sd", name="sd")
                nc.scalar.activation(out=sd, in_=mv[:, 1:2], func=AF.Sqrt,
                                     bias=eps_t, scale=1.0)
                rstd = pd3_o.tile([128, 1], F32, tag="rsd", name="rsd")
                nc.vector.reciprocal(out=rstd, in_=sd)
                nmr = pd3_o.tile([128, 1], F32, tag="nmr", name="nmr")
                nc.vector.scalar_tensor_tensor(
                    out=nmr, in0=mv[:, 0:1], scalar=-1.0, in1=rstd,
                    op0=OP.mult, op1=OP.mult)
                yr = pd3_o.tile([128, H], F32, tag="yr", name="yr")
                for i in range(2):
                    nc.scalar.activation(out=yr[:, i * 512:(i + 1) * 512],
                                         in_=tp[:, i * 512:(i + 1) * 512],
                                         func=AF.Identity, bias=nmr,
                                         scale=rstd)
                if not ln2_trivial:
                    nc.vector.tensor_mul(yr, yr, l2g_bc)
                    nc.vector.tensor_add(yr, yr, l2b_bc)
                nc.sync.dma_start(out=d_out[row0:row0 + 128, :], in_=yr)

            # Pass 1: all gates (only need x8/ctx8) -> sigb, keeps the PE
            # busy while the iLN apply/gelu chain drains on DVE/ACT.
            sigb = pd3.tile([128, HC, R], BF16)
            for rt in range(RT):
                rsl = bass.ts(rt, 512)
                for oc in range(HC):
                    gps = pd3_ps.tile([128, 512], F32, tag="gps", name="gps",
                                      bufs=2)
                    for i in range(4):
                        nc.tensor.matmul(gps, gwx8[:, oc, i],
                                         x8[:, 2 * i:2 * i + 2, rsl],
                                         start=(i == 0), stop=False,
                                         perf_mode=DR)
                    for i in range(4):
                        nc.tensor.matmul(gps, gwa8[:, oc, i],
                                         ctx8[:, 2 * i:2 * i + 2, rsl],
                                         start=False, stop=(i == 3),
                                         perf_mode=DR)
                    nc.scalar.activation(out=sigb[:, oc, rsl], in_=gps,
                                         func=AF.Sigmoid,
                                         bias=gb_pp[:, oc:oc + 1],
                                         scale=1.0 / SGX)

            # Pass 2: integ + residual + LN2, 512-row tiles
            for rt in range(RT):
                rsl = bass.ts(rt, 512)
                yt = pd3_y.tile([128, HC, 512], F32, tag="yt", name="yt")
                for oc in range(HC):
                    w2s = pd3_w.tile([128, H2C, 128], BF16, tag="w2s",
                                     name="w2s")
                    nc.sync.dma_start(out=w2s, in_=d_w2b[:, oc])
                    ips = pd3_ps.tile([128, 512], F32, tag="ips", name="ips",
                                      bufs=2)
                    for hc2 in range(H2C):
                        nc.tensor.matmul(ips, w2s[:, hc2],
                                         h1b[:, hc2, rsl],
                                         start=(hc2 == 0),
                                         stop=(hc2 == H2C - 1))
                    tmp = pd3_t.tile([128, 512], F32, tag="ytmp", name="ytmp")
                    nc.vector.scalar_tensor_tensor(
                        out=tmp, in0=ips, scalar=b2_pp[:, oc:oc + 1],
                        in1=sigb[:, oc, rsl], op0=OP.add, op1=OP.mult)
                    nc.vector.tensor_add(yt[:, oc, :], tmp,
                                         xt[:, oc, rsl].bitcast(F32))
                for rc2 in range(4):
                    d4_chunk(yt, rt, rc2)

    nc.compile()
    return nc


_NC_CACHE = {}


def _get_nc(ln2_trivial):
    if ln2_trivial not in _NC_CACHE:
        _NC_CACHE[ln2_trivial] = build_program(ln2_trivial)
    return _NC_CACHE[ln2_trivial]


def kernel(query_hidden, mem_keys, importance, recency, access_count,
           Wq, bq, in_w, in_b, out_w, out_b, gate_w, gate_b,
           int_w1, int_b1, int_ln_g, int_ln_b, int_w2, int_b2,
           ln1_g, ln1_b, ln2_g, ln2_b, sel_params, top_k):
    np32 = lambda a: np.asarray(a, dtype=np.float32)
    f8 = lambda a: np.ascontiguousarray(a.astype(NP_FP8))
    bf = lambda a: np.ascontiguousarray(a.astype(NP_BF16))

    query_hidden = np32(query_hidden)
    mem_keys = np32(mem_keys)
    top_k = int(top_k)
    assert top_k == K, f"kernel compiled for top_k={K}, got {top_k}"

    # HTPS selection (host): softmax-weighted score, top-k set, gather.
    sp = np32(sel_params)
    w = np.exp(sp - sp.max())
    w = w / w.sum()
    acc = np32(access_count)
    sel = (w[0] * np32(importance) + w[1] * np32(recency)
           + w[2] * (acc / acc.max()))
    idx = np.argpartition(-sel, top_k - 1)[:top_k]
    mem = mem_keys[idx]                               # [K, H]

    mu = mem.mean(1)
    rstd = 1.0 / np.sqrt(mem.var(1) + EPS)

    in_w = np32(in_w)
    in_b = np32(in_b)
    wq, wk, wv = in_w[:H], in_w[H:2 * H], in_w[2 * H:]
    bqi, bki, bvi = in_b[:H], in_b[H:2 * H], in_b[2 * H:]
    g1 = np32(ln1_g)
    b1v = np32(ln1_b)

    wkE = wk * g1[None, :]
    wvE = wv * g1[None, :]
    wc = wq @ np32(Wq)
    bcv = wq @ np32(bq) + bqi

    out_w = np32(out_w)
    gate_w = np32(gate_w)
    int_w1 = np32(int_w1)
    gwx, gwa = gate_w[:, :H], gate_w[:, H:]
    w1x, w1a = int_w1[:, :H], int_w1[:, H:]
    bvF = bvi + wv @ b1v                              # V bias (exact)
    out_bF = out_w @ bvF + np32(out_b)
    gate_bF = np32(gate_b) + gwa @ out_bF
    int_b1F = np32(int_b1) + w1a @ out_bF
    gwaE = gwa @ out_w
    w1aE = w1a @ out_w

    def chunk5(wmat, scale):
        # [O, F] -> fp8 [128, O//128, 4, 2, 128] (pair-sliced stationary)
        o, f = wmat.shape
        a = np.ascontiguousarray(wmat.T) * scale      # [F, O]
        a = a.reshape(4, 2, 128, o // 128, 128)       # f = (2i+j)*128 + p
        return f8(np.ascontiguousarray(a.transpose(2, 3, 0, 1, 4)))

    X = query_hidden.reshape(B * S, H)

    def fm(a, dtype_cast):
        # [N, 128*c] -> feature-major [128, c, N]
        n, ftot = a.shape
        t = np.ascontiguousarray(a.T).reshape(ftot // 128, 128, n)
        return dtype_cast(np.ascontiguousarray(t.transpose(1, 0, 2)))

    # extended mem: chunks 8-9 are the LN-correction pair, row (0,0)=32*mu
    m8e = np.zeros((128, HC + 2, K), dtype=NP_FP8)
    m8e[:, :HC, :] = fm(mem, f8)
    m8e[0, HC, :] = (SKV * mu).astype(NP_FP8)

    wk8e = np.zeros((128, HC, 5, 2, 128), dtype=NP_FP8)
    wk8e[:, :, :4] = chunk5(wkE, SKV)
    wk8e[0, :, 4, 0, :] = (-wkE.sum(1)).astype(NP_FP8).reshape(HC, 128)

    wv8e = np.zeros((128, 5, 2, H), dtype=NP_FP8)
    wv8e[:, :4] = f8(np.ascontiguousarray(
        (wvE.T * SKV).reshape(4, 2, 128, H).transpose(2, 0, 1, 3)))
    wv8e[0, 4, 0, :] = (-wvE.sum(1)).astype(NP_FP8)

    common = {
        "m8": m8e,
        "wk8": wk8e,
        "wc8": chunk5(wc, SQ),
        "wv8": wv8e,
        "gwx8": chunk5(gwx, SGX),
        "gwa8": chunk5(gwaE, SGA),
        "w1a8": chunk5(w1aE, S1A),
        "w1x": np.ascontiguousarray(
            w1x.T.reshape(HC, 128, H2C, 128).transpose(2, 1, 0, 3)),
        "w2b": bf(np.ascontiguousarray(
            np32(int_w2).T.reshape(H2C, 128, HC, 128).transpose(1, 2, 0, 3))),
        "cv": rstd / SKV,
        "bc": bcv,
        "gb": gate_bF,
        "b1": int_b1F,
        "b2": np32(int_b2),
        "ilg": np32(int_ln_g),
        "ilb": np32(int_ln_b),
        "l2g": np32(ln2_g),
        "l2b": np32(ln2_b),
    }

    ln2_trivial = bool(np.all(np32(ln2_g) == 1.0)
                       and np.all(np32(ln2_b) == 0.0))

    in_maps = []
    for c in range(N_CORES):
        m = dict(common)
        Xc = X[c * R:(c + 1) * R]
        m["x8"] = fm(Xc, f8)
        m["xt"] = fm(Xc, lambda a: np.ascontiguousarray(a))
        in_maps.append(m)

    nc = _get_nc(ln2_trivial)
    res = run_bass_kernel_spmd(nc, in_maps, core_ids=list(range(N_CORES)))
    out = np.empty((B * S, H), dtype=np.float32)
    for c in range(N_CORES):
        out[c * R:(c + 1) * R] = res.results[c]["out"]
    return out.reshape(B, S, H)
